# revision 1
# baseline (speedup 1.0000x reference)
"""Trainium2 Bass kernel for nn_ConvolutionalNMPBlock.

Self-contained: takes full (unsharded) inputs, shards batch across 8
NeuronCores (2 elements each), runs a fused Bass/Tile kernel, gathers.
"""
import numpy as np
import ml_dtypes

BS, N, D = 16, 2048, 256
NCORE = 8
PER = BS // NCORE          # batch elements per core
EPS = 1e-5
NB = N // 128              # 16 row blocks
LC = N // 512              # 4 column chunks of 512
DC = D // 128              # 2 channel blocks
KT = 17                    # conv2 taps
BF = ml_dtypes.bfloat16

_built = {}                # use_mask -> compiled nc


def _build(use_mask: bool, use_bias: bool = True, loop_n: int = 1,
           skip: frozenset = frozenset(), body_reps: int = 1):
    from concourse import bacc, tile
    import concourse.mybir as mybir
    from contextlib import ExitStack

    f32 = mybir.dt.float32
    bf16 = mybir.dt.bfloat16
    AF = mybir.ActivationFunctionType
    OP = mybir.AluOpType

    nc = bacc.Bacc("TRN2", target_bir_lowering=False, debug=False,
                   num_devices=NCORE)

    def din(name, shape, dt=f32):
        return nc.dram_tensor(name, shape, dt, kind="ExternalInput").ap()

    x_d = din("x", (PER, N, D))
    w1_d = din("w1t", (128, DC, DC, 128), bf16)        # [p=kin, kc, mc, m]
    w2_d = din("w2t", (128, DC, KT, DC, 128), bf16)    # [p, kc, tap, mc, m]
    sh1_d = din("sh1", (128, DC))
    sh2_d = din("sh2", (128, DC))
    wmsg_d = din("wmsgt", (128, DC, D), bf16)          # [p, kc, f]
    bmsg_d = din("bmsg", (1, D), bf16)
    wseh_d = din("wseth", (128, DC, 3), bf16)
    wsel_d = din("wsetl", (128, DC, 3), bf16)
    bse_d = din("bse", (1, 3), bf16)
    wih_d = din("wiht", (128, 4, 3 * D), bf16)         # [p, kc, f]
    whh_d = din("whht", (128, DC, 3 * D), bf16)
    brz_d = din("brow_rz", (1, 2 * D), bf16)
    bgin_d = din("brow_gin", (1, D), bf16)
    bghn_d = din("brow_ghn", (1, D), bf16)
    ones_d = din("ones128", (1, 128), bf16)
    ones5_d = din("ones512", (1, 512), bf16)
    ones2n_d = din("ones2n", (2, N), bf16)
    if use_mask:
        mt_d = din("maskt", (PER, N, N), bf16)
    out_d = nc.dram_tensor("out", (PER, N, D), f32, kind="ExternalOutput").ap()

    with tile.TileContext(nc) as tc, ExitStack() as ctx:
        if loop_n > 1:
            ctx.enter_context(tc.For_i(0, loop_n, 1))
        wp = ctx.enter_context(tc.tile_pool(name="wp", bufs=1))
        big = ctx.enter_context(tc.tile_pool(name="big", bufs=1))
        rawp = ctx.enter_context(tc.tile_pool(name="rawp", bufs=1))
        atp = ctx.enter_context(tc.tile_pool(name="atp", bufs=33))
        cvp = ctx.enter_context(tc.tile_pool(name="cvp", bufs=2))
        gtp = ctx.enter_context(tc.tile_pool(name="gtp", bufs=1))
        dramp = ctx.enter_context(tc.tile_pool(name="dramp", bufs=1, space="DRAM"))
        ps = ctx.enter_context(tc.tile_pool(name="ps", bufs=4, space="PSUM"))
        pa = ctx.enter_context(tc.tile_pool(name="pa", bufs=2, space="PSUM"))

        # ---- load weights (persistent) ----
        w1 = wp.tile([128, DC, DC, 128], bf16, tag="w1")
        nc.gpsimd.dma_start(w1[:], w1_d[:])
        sh1 = wp.tile([128, DC], f32, tag="sh1")
        nc.gpsimd.dma_start(sh1[:], sh1_d[:])
        sh2 = wp.tile([128, DC], f32, tag="sh2")
        nc.gpsimd.dma_start(sh2[:], sh2_d[:])
        bmsg = wp.tile([1, D], bf16, tag="bmsg")
        nc.gpsimd.dma_start(bmsg[:], bmsg_d[:])
        wseh = wp.tile([128, DC, 3], bf16, tag="wseh")
        nc.gpsimd.dma_start(wseh[:], wseh_d[:])
        wsel = wp.tile([128, DC, 3], bf16, tag="wsel")
        nc.gpsimd.dma_start(wsel[:], wsel_d[:])
        bse = wp.tile([1, 3], bf16, tag="bse")
        nc.gpsimd.dma_start(bse[:], bse_d[:])
        ones = wp.tile([1, 128], bf16, tag="ones")
        nc.gpsimd.dma_start(ones[:], ones_d[:])
        ones5 = wp.tile([1, 512], bf16, tag="ones5")
        nc.gpsimd.dma_start(ones5[:], ones5_d[:])

        for el in [e for _ in range(body_reps) for e in range(PER)]:
            # ---- load x n-major (contiguous), split bf16 hi/lo, and build
            # channel-major copies via the XBAR transpose DMA (2-byte dtype).
            xnf = rawp.tile([128, NB, D], f32, tag="xnf", bufs=2)
            xnh = rawp.tile([128, NB, D], bf16, tag="xnh", bufs=1)
            xnl = rawp.tile([128, NB, D], bf16, tag="xnl", bufs=1)
            # layout: xth[p_c, nb, dc, p_n] == xT[dc*128+p_c, nb*128+p_n]
            xth = big.tile([128, NB, DC, 128], bf16, tag="xth", bufs=2)
            xtl = big.tile([128, NB, DC, 128], bf16, tag="xtl", bufs=1)
            qn = NB // 4
            quarters = [slice(h * qn, (h + 1) * qn) for h in range(4)]
            for h_, hs in enumerate(quarters):
                nc.sync.dma_start(
                    xnf[:, hs, :],
                    x_d[el, h_ * (N // 4):(h_ + 1) * (N // 4), :].rearrange(
                        "(nb p) d -> p nb d", p=128))
            for hs in quarters:
                nc.vector.tensor_copy(xnh[:, hs, :], xnf[:, hs, :])
                nc.vector.scalar_tensor_tensor(xnl[:, hs, :], xnf[:, hs, :],
                                               1.0, xnh[:, hs, :],
                                               OP.mult, OP.subtract)
            for hs in quarters:
                nc.scalar.dma_start_transpose(
                    xth[:, hs].rearrange("p nb dc pn -> p (nb dc) pn"),
                    xnh[:, hs, :])
            for hs in quarters:
                nc.scalar.dma_start_transpose(
                    xtl[:, hs].rearrange("p nb dc pn -> p (nb dc) pn"),
                    xnl[:, hs, :])

            if el == 0:
                w2 = wp.tile([128, DC, KT, DC, 128], bf16, tag="w2")
                nc.sync.dma_start(w2[:], w2_d[:])
                wmsg = wp.tile([128, DC, D], bf16, tag="wmsg")
                nc.sync.dma_start(wmsg[:], wmsg_d[:])
                wih = wp.tile([128, 4, 3 * D], bf16, tag="wih")
                nc.sync.dma_start(wih[:], wih_d[:])
                whh = wp.tile([128, DC, 3 * D], bf16, tag="whh")
                nc.sync.dma_start(whh[:], whh_d[:])
                brz = wp.tile([1, 2 * D], bf16, tag="brz")
                nc.sync.dma_start(brz[:], brz_d[:])
                bgin = wp.tile([1, D], bf16, tag="bgin")
                nc.sync.dma_start(bgin[:], bgin_d[:])
                bghn = wp.tile([1, D], bf16, tag="bghn")
                nc.sync.dma_start(bghn[:], bghn_d[:])
            # ---- s = x @ w_se.T + b_se  (f32-accurate via hi/lo) ----
            st = big.tile([3, N], f32, tag="st", bufs=1)
            for c in range(LC):
                pss = ps.tile([3, 512], f32, tag="ps")
                first = True
                for kc in range(DC):
                    for wi_, (wse_, xt_) in enumerate(
                            ((wseh, xth), (wseh, xtl), (wsel, xth))):
                        last = (not use_bias) and kc == DC - 1 and wi_ == 2
                        nc.tensor.matmul(pss[:], wse_[:, kc, :],
                                         xt_[:, 4 * c:4 * (c + 1), kc, :],
                                         start=first, stop=last,
                                         skip_group_check=True)
                        first = False
                if use_bias:
                    nc.tensor.matmul(pss[:], bse[:], ones5[:], start=False,
                                     stop=True, skip_group_check=True)
                nc.vector.tensor_copy(st[:, c * 512:(c + 1) * 512], pss[:])

            # ---- augmented 13-row factors for exp(-dist) matmul ----
            # k-row pairs (SA | SB): 0:3 (2s_hi | s_hi), 3:6 (2s_lo | s_hi),
            # 6:9 (2s_hi | s_lo), 9,10 (1 | -sq_hi, -sq_lo),
            # 11,12 (-sq_hi, -sq_lo | 1).
            # Built in a WIDE (128-partition) layout: st (3, N) costs ~2k DVE
            # cycles per op on 3 lanes, so bounce to (128, 3, NB) via DRAM,
            # do all the arithmetic at 128-lane width, and bounce the
            # finished 13-row stacks back.  w[p, r, g] == row[r, g*128+p].
            scst = dramp.tile([3, N], f32, tag="scst", bufs=1)
            nc.sync.dma_start(scst[:], st[:])
            stw = cvp.tile([128, 3, NB], f32, tag="stw", bufs=1)
            nc.sync.dma_start(stw[:], scst[:].rearrange("c (g p) -> p c g", p=128))
            ssqw = cvp.tile([128, 3, NB], f32, tag="ssqw", bufs=1)
            nc.scalar.activation(ssqw[:], stw[:], AF.Square)
            sqw = cvp.tile([128, NB], f32, tag="sqw", bufs=1)
            nc.vector.tensor_tensor(sqw[:], ssqw[:, 0, :], ssqw[:, 1, :], OP.add)
            nc.vector.tensor_tensor(sqw[:], sqw[:], ssqw[:, 2, :], OP.add)

            saw = cvp.tile([128, 13, NB], bf16, tag="saw", bufs=1)
            sbw = cvp.tile([128, 13, NB], bf16, tag="sbw", bufs=1)
            nc.vector.tensor_scalar(saw[:, 0:3, :], stw[:], 2.0, None, OP.mult)
            nc.vector.scalar_tensor_tensor(saw[:, 3:6, :], stw[:], 2.0,
                                           saw[:, 0:3, :], OP.mult, OP.subtract)
            nc.vector.tensor_copy(saw[:, 6:9, :], saw[:, 0:3, :])
            nc.vector.memset(saw[:, 9:11, :], 1.0)
            nc.vector.tensor_scalar(saw[:, 11:12, :], sqw[:].unsqueeze(1),
                                    -1.0, None, OP.mult)
            nc.vector.scalar_tensor_tensor(saw[:, 12:13, :], sqw[:].unsqueeze(1),
                                           -1.0, saw[:, 11:12, :],
                                           OP.mult, OP.subtract)
            nc.vector.tensor_copy(sbw[:, 0:3, :], stw[:])
            nc.vector.tensor_copy(sbw[:, 3:6, :], sbw[:, 0:3, :])
            nc.vector.scalar_tensor_tensor(sbw[:, 6:9, :], stw[:], 1.0,
                                           sbw[:, 0:3, :], OP.mult, OP.subtract)
            nc.vector.tensor_copy(sbw[:, 9:11, :], saw[:, 11:13, :])
            nc.vector.memset(sbw[:, 11:13, :], 1.0)

            sa = big.tile([13, N], bf16, tag="sa", bufs=1)
            sb = big.tile([13, N], bf16, tag="sb", bufs=1)
            scsa = dramp.tile([13, N], bf16, tag="scsa", bufs=1)
            scsb = dramp.tile([13, N], bf16, tag="scsb", bufs=1)
            nc.sync.dma_start(scsa[:].rearrange("r (g p) -> p r g", p=128), saw[:])
            nc.sync.dma_start(sa[:], scsa[:])
            nc.sync.dma_start(scsb[:].rearrange("r (g p) -> p r g", p=128), sbw[:])
            nc.sync.dma_start(sb[:], scsb[:])

            # ---- conv1 (1x1) + bn1 + relu -> h1 (padded by 8 each side) ----
            h1 = big.tile([128, DC, N + 16], bf16, tag="h1", bufs=1)
            nc.vector.memset(h1[:, :, 0:8], 0.0)
            nc.vector.memset(h1[:, :, N + 8:N + 16], 0.0)
            for mc in range(DC):
                for c in range(LC):
                    pc = ps.tile([128, 512], f32, tag="ps")
                    for kc in range(DC):
                        nc.tensor.matmul(pc[:], w1[:, kc, mc, :],
                                         xth[:, 4 * c:4 * (c + 1), kc, :],
                                         start=(kc == 0), stop=(kc == DC - 1),
                                         skip_group_check=True)
                    nc.scalar.activation(h1[:, mc, 8 + c * 512:8 + (c + 1) * 512],
                                         pc[:], AF.Relu, bias=sh1[:, mc:mc + 1])

            # ---- conv2 (17 taps) + bn2 + residual + relu -> x_convT ----
            xcv = big.tile([128, DC, N], bf16, tag="xcv")
            if "conv2" in skip:
                nc.vector.memset(xcv[:], 0.0)
            for mc in range(0 if "conv2" in skip else DC):
                for c in range(LC):
                    pc2 = ps.tile([128, 512], f32, tag="ps")
                    first = True
                    for kc in range(DC):
                        for t in range(KT):
                            nc.tensor.matmul(
                                pc2[:], w2[:, kc, t, mc, :],
                                h1[:, kc, c * 512 + t:c * 512 + t + 512],
                                start=first, stop=(kc == DC - 1 and t == KT - 1),
                                skip_group_check=True)
                            first = False
                    tv = cvp.tile([128, 4, 128], f32, tag="cv", bufs=2)
                    nc.vector.tensor_tensor(tv[:], pc2[:].rearrange(
                        "p (a b) -> p a b", b=128),
                        xth[:, 4 * c:4 * (c + 1), mc, :], OP.add)
                    nc.vector.tensor_tensor(tv[:], tv[:],
                                            xtl[:, 4 * c:4 * (c + 1), mc, :],
                                            OP.add)
                    nc.scalar.activation(
                        xcv[:, mc, c * 512:(c + 1) * 512],
                        tv[:].rearrange("p a b -> p (a b)"),
                        AF.Relu, bias=sh2[:, mc:mc + 1])

            # ---- msg = relu(x @ w_msg.T + b_msg), n-major ----
            msg = big.tile([128, NB, D], bf16, tag="msg")
            for nb in range(NB):
                pm = ps.tile([128, 512], f32, tag="ps")
                for kc in range(DC):
                    nc.tensor.matmul(pm[:, 0:D], xth[:, nb, kc, :],
                                     wmsg[:, kc, :], start=(kc == 0),
                                     stop=(not use_bias and kc == DC - 1),
                                     skip_group_check=True)
                if use_bias:
                    nc.tensor.matmul(pm[:, 0:D], ones[:], bmsg[:], start=False,
                                     stop=True, skip_group_check=True)
                nc.scalar.activation(msg[:, nb, :], pm[:, 0:D], AF.Relu)

            # ---- A-branch: x_nmpT[d, i] = sum_j exp(-dist[j,i]) * msg[j, d] ----
            xnm = big.tile([128, DC, N], bf16, tag="xnm")
            if "noG" in skip and el == 0:
                atc = wp.tile([128, 512], bf16, tag="atc")
                nc.vector.memset(atc[:], 0.001)
            if "A" in skip:
                nc.vector.memset(xnm[:], 0.0)

            # A-branch restructured into long clean matmul runs (conv2-style):
            # per i-chunk, batch all 16 G matmuls + exps, then run the 16
            # m0-accumulations back-to-back into one bank, then the 16 m1 —
            # software-pipelined one chunk deep (acc of chunk ic-1 overlaps
            # exp of chunk ic).  Interleaved short groups measured ~560 ns/MM
            # on HW vs ~240 ns/MM for long runs.
            at_store = {}

            def emit_acc(ic):
                accs = [pa.tile([128, 512], f32, tag="acc0", name="a0"),
                        pa.tile([128, 512], f32, tag="acc1", name="a1")]
                for mc in range(DC):
                    for jb in range(NB):
                        nc.tensor.matmul(accs[mc][:],
                                         msg[:, jb, mc * 128:(mc + 1) * 128],
                                         at_store[(ic, jb)][:],
                                         start=(jb == 0), stop=(jb == NB - 1),
                                         skip_group_check=True)
                for mc in range(DC):
                    nc.vector.tensor_copy(xnm[:, mc, ic * 512:(ic + 1) * 512],
                                          accs[mc][:])

            for ic in range(0 if "A" in skip else LC):
                for jb in range(NB):
                    if "noG" in skip:
                        at_store[(ic, jb)] = atc
                        continue
                    pgm = ps.tile([128, 512], f32, tag="ps")
                    nc.tensor.matmul(pgm[:], sa[:, jb * 128:(jb + 1) * 128],
                                     sb[:, ic * 512:(ic + 1) * 512],
                                     start=True, stop=True,
                                     skip_group_check=True)
                    at = atp.tile([128, 512], bf16, tag="at")
                    if "exp2dve" in skip:
                        nc.vector.tensor_copy(at[:], pgm[:])
                    else:
                        nc.scalar.activation(at[:], pgm[:], AF.Exp)
                    if use_mask:
                        mtt = cvp.tile([128, 512], bf16, tag="mtt")
                        nc.sync.dma_start(mtt[:],
                                          mt_d[el, jb * 128:(jb + 1) * 128,
                                               ic * 512:(ic + 1) * 512])
                        nc.vector.tensor_tensor(at[:], at[:], mtt[:], OP.mult)
                    at_store[(ic, jb)] = at
                if ic >= 1:
                    emit_acc(ic - 1)
            if "A" not in skip:
                emit_acc(LC - 1)

            # ---- GRU gates (n-major) ----
            for nb in range(NB):
                sl = slice(nb * 128, (nb + 1) * 128)
                prz = ps.tile([128, 512], f32, tag="ps")
                ih_srcs = [xcv[:, 0, sl], xcv[:, 1, sl], xnm[:, 0, sl], xnm[:, 1, sl]]
                hh_srcs = [xth[:, nb, 0, :], xth[:, nb, 1, :]]
                for ci in range(4):
                    nc.tensor.matmul(prz[:], ih_srcs[ci], wih[:, ci, 0:512],
                                     start=(ci == 0), stop=False,
                                     skip_group_check=True)
                for kc in range(DC):
                    nc.tensor.matmul(prz[:], hh_srcs[kc], whh[:, kc, 0:512],
                                     start=False,
                                     stop=(not use_bias and kc == DC - 1),
                                     skip_group_check=True)
                if use_bias:
                    nc.tensor.matmul(prz[:], ones[:], brz[:], start=False,
                                     stop=True, skip_group_check=True)

                # gi_n in cols 0:D, gh_n in cols D:2D of ONE psum tile —
                # 2 psum allocs per block instead of 3, so two blocks pipeline
                pgg = ps.tile([128, 512], f32, tag="ps")
                for ci in range(4):
                    nc.tensor.matmul(pgg[:, 0:D], ih_srcs[ci], wih[:, ci, 512:768],
                                     start=(ci == 0),
                                     stop=(not use_bias and ci == 3),
                                     skip_group_check=True)
                if use_bias:
                    nc.tensor.matmul(pgg[:, 0:D], ones[:], bgin[:], start=False,
                                     stop=True, skip_group_check=True)
                for kc in range(DC):
                    nc.tensor.matmul(pgg[:, D:2 * D], hh_srcs[kc],
                                     whh[:, kc, 512:768], start=(kc == 0),
                                     stop=(not use_bias and kc == DC - 1),
                                     skip_group_check=True)
                if use_bias:
                    nc.tensor.matmul(pgg[:, D:2 * D], ones[:], bghn[:],
                                     start=False, stop=True,
                                     skip_group_check=True)

                tr = gtp.tile([128, D], f32, tag="tr")
                nc.scalar.activation(tr[:], prz[:, 0:D], AF.Tanh, scale=0.5)
                tz = gtp.tile([128, D], f32, tag="tz")
                nc.scalar.activation(tz[:], prz[:, D:2 * D], AF.Tanh, scale=0.5)
                # r,z in place; q accumulates in place; ee overwrites dd
                nc.vector.tensor_scalar(tz[:], tz[:], 0.5, 0.5, OP.mult, OP.add)
                nc.vector.tensor_scalar(tr[:], tr[:], 0.5, 0.5, OP.mult, OP.add)
                q = gtp.tile([128, D], f32, tag="q")
                nc.vector.tensor_tensor(q[:], tr[:], pgg[:, D:2 * D], OP.mult)
                nc.vector.tensor_tensor(q[:], q[:], pgg[:, 0:D], OP.add)
                nn = gtp.tile([128, D], f32, tag="nn")
                nc.scalar.activation(nn[:], q[:], AF.Tanh)
                dd = gtp.tile([128, D], f32, tag="dd")
                nc.vector.tensor_tensor(dd[:], xnf[:, nb, :], nn[:], OP.subtract)
                nc.vector.tensor_tensor(dd[:], tz[:], dd[:], OP.mult)
                ho = gtp.tile([128, D], f32, tag="ho", bufs=2)
                nc.vector.tensor_tensor(ho[:], nn[:], dd[:], OP.add)
                nc.sync.dma_start(out_d[el, sl, :], ho[:])

    nc.compile()
    return nc


def _host_prep(inputs):
    g = {k: np.asarray(v, np.float32) for k, v in inputs.items()}
    sc1 = g["bn1_g"] / np.sqrt(g["bn1_v"] + EPS)
    sh1 = g["bn1_b"] - g["bn1_m"] * sc1
    sc2 = g["bn2_g"] / np.sqrt(g["bn2_v"] + EPS)
    sh2 = g["bn2_b"] - g["bn2_m"] * sc2

    w1p = g["conv1_w"][:, :, 0] * sc1[:, None]          # (O, I)
    w2p = g["conv2_w"] * sc2[:, None, None]             # (O, I, 17)

    def lhsT_pack(w):   # (O, I) -> (128, kc=I/128, mc=O/128, 128): [p,kc,mc,m]
        o, i = w.shape
        return np.ascontiguousarray(np.transpose(
            w.T.reshape(i // 128, 128, o // 128, 128), (1, 0, 2, 3)))

    w1t = lhsT_pack(w1p).astype(BF)
    w2t = np.stack([lhsT_pack(w2p[:, :, t]) for t in range(KT)], axis=2)
    w2t = np.ascontiguousarray(np.transpose(w2t, (0, 1, 2, 3, 4)))  # [p,kc,t,mc,m]
    w2t = w2t.astype(BF)

    def rhs_pack(wt):   # (Kdim, F) -> (128, kc, F)
        k, f = wt.shape
        return np.ascontiguousarray(
            np.transpose(wt.reshape(k // 128, 128, f), (1, 0, 2)))

    wmsgt = rhs_pack(g["w_msg"].T).astype(BF)
    wiht = rhs_pack(g["w_ih"].T).astype(BF)
    whht = rhs_pack(g["w_hh"].T).astype(BF)

    wse_t = g["w_se"].T                                  # (256, 3)
    wse_hi = wse_t.astype(BF)
    wse_lo = (wse_t - wse_hi.astype(np.float32)).astype(BF)
    wseth = rhs_pack(wse_hi.astype(np.float32)).astype(BF)
    wsetl = rhs_pack(wse_lo.astype(np.float32)).astype(BF)

    bih, bhh = g["b_ih"], g["b_hh"]
    feed = {
        "w1t": w1t, "w2t": w2t,
        "sh1": np.ascontiguousarray(sh1.reshape(DC, 128).T.astype(np.float32)),
        "sh2": np.ascontiguousarray(sh2.reshape(DC, 128).T.astype(np.float32)),
        "wmsgt": wmsgt, "bmsg": g["b_msg"].reshape(1, D).astype(BF),
        "wseth": wseth, "wsetl": wsetl,
        "bse": g["b_se"].reshape(1, 3).astype(BF),
        "wiht": wiht, "whht": whht,
        "brow_rz": (bih[:2 * D] + bhh[:2 * D]).reshape(1, 2 * D).astype(BF),
        "brow_gin": bih[2 * D:].reshape(1, D).astype(BF),
        "brow_ghn": bhh[2 * D:].reshape(1, D).astype(BF),
        "ones128": np.ones((1, 128), BF),
        "ones512": np.ones((1, 512), BF),
        "ones2n": np.ones((2, N), BF),
    }
    return g, feed


def make_in_maps(inputs):
    g, feed = _host_prep(inputs)
    x = g["x"]
    mask = g["mask"]
    use_mask = not bool(np.all(mask == 1.0))
    use_bias = not (np.all(g["b_se"] == 0) and np.all(g["b_msg"] == 0)
                    and np.all(g["b_ih"] == 0) and np.all(g["b_hh"] == 0))
    in_maps = []
    for i in range(NCORE):
        m = dict(feed)
        m["x"] = np.ascontiguousarray(x[i * PER:(i + 1) * PER])
        if use_mask:
            m["maskt"] = np.ascontiguousarray(
                mask[i * PER:(i + 1) * PER].transpose(0, 2, 1)).astype(BF)
        in_maps.append(m)
    return in_maps, use_mask, use_bias


def get_nc(use_mask: bool, use_bias: bool = True):
    key = (use_mask, use_bias)
    if key not in _built:
        _built[key] = _build(use_mask, use_bias)
    return _built[key]


def kernel(**inputs) -> np.ndarray:
    in_maps, use_mask, use_bias = make_in_maps(inputs)
    nc = get_nc(use_mask, use_bias)
    from concourse import bass_utils
    last_err = None
    for attempt in range(3):
        try:
            res = bass_utils.run_bass_kernel_spmd(nc, in_maps,
                                                  core_ids=list(range(NCORE)))
            out = np.concatenate([res.results[i]["out"] for i in range(NCORE)],
                                 axis=0)
            return np.ascontiguousarray(out.astype(np.float32))
        except Exception as e:  # wedged device: reset backend and retry
            last_err = e
            try:
                import jax
                jax.clear_caches()
                jax.extend.backend.clear_backends()
            except Exception:
                pass
            import time as _t
            _t.sleep(5)
    raise last_err



# revision 18
# speedup vs baseline: 1.4563x; 1.4563x over previous
"""Trainium2 Bass kernel for nn_ConvolutionalNMPBlock.

Self-contained: takes full (unsharded) inputs, shards batch across 8
NeuronCores (2 elements each), runs a fused Bass/Tile kernel, gathers.
"""
import numpy as np
import ml_dtypes

BS, N, D = 16, 2048, 256
NCORE = 8
PER = BS // NCORE          # batch elements per core
EPS = 1e-5
NB = N // 128              # 16 row blocks
LC = N // 512              # 4 column chunks of 512
DC = D // 128              # 2 channel blocks
KT = 17                    # conv2 taps
BF = ml_dtypes.bfloat16

_built = {}                # use_mask -> compiled nc


def _build(use_mask: bool, use_bias: bool = True, loop_n: int = 1,
           skip: frozenset = frozenset(), body_reps: int = 1):
    from concourse import bacc, tile
    import concourse.mybir as mybir
    from contextlib import ExitStack

    f32 = mybir.dt.float32
    bf16 = mybir.dt.bfloat16
    AF = mybir.ActivationFunctionType
    OP = mybir.AluOpType

    nc = bacc.Bacc("TRN2", target_bir_lowering=False, debug=False,
                   num_devices=NCORE)

    def din(name, shape, dt=f32):
        return nc.dram_tensor(name, shape, dt, kind="ExternalInput").ap()

    x_d = din("x", (PER, N, D))
    w1_d = din("w1t", (128, DC, DC, 128), bf16)        # [p=kin, kc, mc, m]
    w2_d = din("w2t", (128, DC, KT, DC, 128), bf16)    # [p, kc, tap, mc, m]
    sh1_d = din("sh1", (128, DC))
    sh2_d = din("sh2", (128, DC))
    wmsg_d = din("wmsgt", (128, DC, D), bf16)          # [p, kc, f]
    bmsg_d = din("bmsg", (1, D), bf16)
    wsehl_d = din("wsehl", (128, DC, 6), bf16)         # [p, kc, hi(3)|lo(3)]
    bse_d = din("bse", (1, 3), bf16)
    wih_d = din("wiht", (128, 4, 3 * D), bf16)         # [p, kc, f]
    whh_d = din("whht", (128, DC, 3 * D), bf16)
    brz_d = din("brow_rz", (1, 2 * D), bf16)
    bgin_d = din("brow_gin", (1, D), bf16)
    bghn_d = din("brow_ghn", (1, D), bf16)
    ones_d = din("ones128", (1, 128), bf16)
    ones5_d = din("ones512", (1, 512), bf16)
    ones2n_d = din("ones2n", (2, N), bf16)
    if use_mask:
        mt_d = din("maskt", (PER, N, N), bf16)
    out_d = nc.dram_tensor("out", (PER, N, D), f32, kind="ExternalOutput").ap()

    with tile.TileContext(nc) as tc, ExitStack() as ctx:
        if loop_n > 1:
            ctx.enter_context(tc.For_i(0, loop_n, 1))
        wp = ctx.enter_context(tc.tile_pool(name="wp", bufs=1))
        big = ctx.enter_context(tc.tile_pool(name="big", bufs=1))
        rawp = ctx.enter_context(tc.tile_pool(name="rawp", bufs=1))
        atp = ctx.enter_context(tc.tile_pool(name="atp", bufs=33))
        cvp = ctx.enter_context(tc.tile_pool(name="cvp", bufs=2))
        gtp = ctx.enter_context(tc.tile_pool(name="gtp", bufs=1))
        dramp = ctx.enter_context(tc.tile_pool(name="dramp", bufs=1, space="DRAM"))
        ps = ctx.enter_context(tc.tile_pool(name="ps", bufs=4, space="PSUM"))
        pa = ctx.enter_context(tc.tile_pool(name="pa", bufs=2, space="PSUM"))

        # ---- load weights (persistent) ----
        w1 = wp.tile([128, DC, DC, 128], bf16, tag="w1")
        nc.gpsimd.dma_start(w1[:], w1_d[:])
        sh1 = wp.tile([128, DC], f32, tag="sh1")
        nc.gpsimd.dma_start(sh1[:], sh1_d[:])
        sh2 = wp.tile([128, DC], f32, tag="sh2")
        nc.gpsimd.dma_start(sh2[:], sh2_d[:])
        bmsg = wp.tile([1, D], bf16, tag="bmsg")
        nc.gpsimd.dma_start(bmsg[:], bmsg_d[:])
        wsehl = wp.tile([128, DC, 6], bf16, tag="wsehl")
        nc.gpsimd.dma_start(wsehl[:], wsehl_d[:])
        bse = wp.tile([1, 3], bf16, tag="bse")
        nc.gpsimd.dma_start(bse[:], bse_d[:])
        ones = wp.tile([1, 128], bf16, tag="ones")
        nc.gpsimd.dma_start(ones[:], ones_d[:])
        ones5 = wp.tile([1, 512], bf16, tag="ones5")
        nc.gpsimd.dma_start(ones5[:], ones5_d[:])
        # n-major sources for the 13-row RBF factors; rows live in cols 0:13,
        # cols 13:128 zeroed once here (transposed junk would land in unused
        # partitions anyway, but keep the sim's finite-checks happy).
        saSRC = wp.tile([128, NB, 128], bf16, tag="saSRC")
        nc.vector.memset(saSRC[:], 0.0)
        nc.vector.memset(saSRC[:, :, 9:11], 1.0)
        sbSRC = wp.tile([128, NB, 128], bf16, tag="sbSRC")
        nc.vector.memset(sbSRC[:], 0.0)
        nc.vector.memset(sbSRC[:, :, 11:13], 1.0)

        for el in [e for _ in range(body_reps) for e in range(PER)]:
            # ---- load x n-major (contiguous), split bf16 hi/lo, and build
            # channel-major copies via the XBAR transpose DMA (2-byte dtype).
            xnf = rawp.tile([128, NB, D], f32, tag="xnf", bufs=2)
            xnh = rawp.tile([128, NB, D], bf16, tag="xnh", bufs=1)
            xnl = rawp.tile([128, NB, D], bf16, tag="xnl", bufs=1)
            # layout: xth[p_c, nb, dc, p_n] == xT[dc*128+p_c, nb*128+p_n]
            xth = big.tile([128, NB, DC, 128], bf16, tag="xth", bufs=2)
            xtl = big.tile([128, NB, DC, 128], bf16, tag="xtl", bufs=1)
            qn = NB // 4
            quarters = [slice(h * qn, (h + 1) * qn) for h in range(4)]
            for h_, hs in enumerate(quarters):
                nc.sync.dma_start(
                    xnf[:, hs, :],
                    x_d[el, h_ * (N // 4):(h_ + 1) * (N // 4), :].rearrange(
                        "(nb p) d -> p nb d", p=128))
            for hs in quarters:
                nc.vector.tensor_copy(xnh[:, hs, :], xnf[:, hs, :])
                nc.vector.scalar_tensor_tensor(xnl[:, hs, :], xnf[:, hs, :],
                                               1.0, xnh[:, hs, :],
                                               OP.mult, OP.subtract)
            # transposes on the sync HWDGE queue — on the scalar (Act) queue
            # they'd serialize behind the previous element's exp/tanh stream.
            for hs in quarters:
                nc.sync.dma_start_transpose(
                    xth[:, hs].rearrange("p nb dc pn -> p (nb dc) pn"),
                    xnh[:, hs, :])
            for hs in quarters:
                nc.sync.dma_start_transpose(
                    xtl[:, hs].rearrange("p nb dc pn -> p (nb dc) pn"),
                    xnl[:, hs, :])

            if el == 0:
                w2 = wp.tile([128, DC, KT, DC, 128], bf16, tag="w2")
                nc.sync.dma_start(w2[:], w2_d[:])
                wmsg = wp.tile([128, DC, D], bf16, tag="wmsg")
                nc.sync.dma_start(wmsg[:], wmsg_d[:])
                wih = wp.tile([128, 4, 3 * D], bf16, tag="wih")
                nc.sync.dma_start(wih[:], wih_d[:])
                whh = wp.tile([128, DC, 3 * D], bf16, tag="whh")
                nc.sync.dma_start(whh[:], whh_d[:])
                brz = wp.tile([1, 2 * D], bf16, tag="brz")
                nc.sync.dma_start(brz[:], brz_d[:])
                bgin = wp.tile([1, D], bf16, tag="bgin")
                nc.sync.dma_start(bgin[:], bgin_d[:])
                bghn = wp.tile([1, D], bf16, tag="bghn")
                nc.sync.dma_start(bghn[:], bghn_d[:])
            # ---- s = x @ w_se.T + b_se computed n-major (f32 via hi/lo),
            # 13-row RBF factors assembled wide (128 lanes, free-dim slices
            # only — compute engines can't address odd partition bases), then
            # XBAR-transposed to the [13 rows, n] layout the G matmul needs.
            # Row pairs (SA | SB): 0:3 (2s_hi | s_hi), 3:6 (2s_lo | s_hi),
            # 6:9 (2s_hi | s_lo), 9,10 (1 | -sq_hi, -sq_lo),
            # 11,12 (-sq_hi, -sq_lo | 1).
            saT = big.tile([128, NB, 128], bf16, tag="saT", bufs=1)
            sbT = big.tile([128, NB, 128], bf16, tag="sbT", bufs=1)
            if "se" in skip:
                nc.vector.memset(saT[:], 0.001)
                nc.vector.memset(sbT[:], 0.001)
            else:
                pss = ps.tile([128, 512], f32, tag="ps")
                for nb in range(NB):
                    c6 = slice(6 * nb, 6 * nb + 6)
                    c3 = slice(6 * nb, 6 * nb + 3)
                    nc.tensor.matmul(pss[:, c6], xth[:, nb, 0, :],
                                     wsehl[:, 0, :], start=True, stop=False,
                                     skip_group_check=True)
                    nc.tensor.matmul(pss[:, c6], xth[:, nb, 1, :],
                                     wsehl[:, 1, :], start=False, stop=False,
                                     skip_group_check=True)
                    nc.tensor.matmul(pss[:, c3], xtl[:, nb, 0, :],
                                     wsehl[:, 0, 0:3], start=False, stop=False,
                                     skip_group_check=True)
                    nc.tensor.matmul(pss[:, c3], xtl[:, nb, 1, :],
                                     wsehl[:, 1, 0:3], start=False,
                                     stop=not use_bias, skip_group_check=True)
                    if use_bias:
                        nc.tensor.matmul(pss[:, c3], ones[:], bse[:],
                                         start=False, stop=True,
                                         skip_group_check=True)
                pssv = pss[:, 0:6 * NB].rearrange("p (nb c) -> p nb c", c=6)
                snm = cvp.tile([128, NB, 3], f32, tag="snm", bufs=1)
                nc.vector.tensor_copy(snm[:], pssv[:, :, 3:6])
                nc.vector.tensor_tensor(snm[:], snm[:], pssv[:, :, 0:3], OP.add)
                shi = cvp.tile([128, NB, 3], bf16, tag="shi", bufs=1)
                nc.vector.tensor_copy(shi[:], snm[:])
                slo = cvp.tile([128, NB, 3], bf16, tag="slo", bufs=1)
                nc.vector.scalar_tensor_tensor(slo[:], snm[:], 1.0, shi[:],
                                               OP.mult, OP.subtract)
                ssq = cvp.tile([128, NB, 3], f32, tag="ssq", bufs=1)
                nc.scalar.activation(ssq[:], snm[:], AF.Square)
                sqn = cvp.tile([128, NB, 1], f32, tag="sqn", bufs=1)
                nc.vector.tensor_tensor(sqn[:], ssq[:, :, 0:1], ssq[:, :, 1:2],
                                        OP.add)
                nc.vector.tensor_tensor(sqn[:], sqn[:], ssq[:, :, 2:3], OP.add)
                nc.vector.tensor_scalar(saSRC[:, :, 0:3], shi[:], 2.0, None,
                                        OP.mult)
                nc.vector.tensor_scalar(saSRC[:, :, 3:6], slo[:], 2.0, None,
                                        OP.mult)
                nc.vector.tensor_copy(saSRC[:, :, 6:9], saSRC[:, :, 0:3])
                nc.vector.tensor_scalar(saSRC[:, :, 11:12], sqn[:], -1.0, None,
                                        OP.mult)
                nc.vector.scalar_tensor_tensor(saSRC[:, :, 12:13], sqn[:], -1.0,
                                               saSRC[:, :, 11:12],
                                               OP.mult, OP.subtract)
                nc.vector.tensor_copy(sbSRC[:, :, 0:3], shi[:])
                nc.vector.tensor_copy(sbSRC[:, :, 3:6], shi[:])
                nc.vector.tensor_copy(sbSRC[:, :, 6:9], slo[:])
                nc.vector.tensor_copy(sbSRC[:, :, 9:11], saSRC[:, :, 11:13])
                nc.sync.dma_start_transpose(
                    saT[:], saSRC[:].rearrange("p nb c -> p (nb c)"))
                nc.sync.dma_start_transpose(
                    sbT[:], sbSRC[:].rearrange("p nb c -> p (nb c)"))

            # ---- conv1 (1x1) + bn1 + relu -> h1 (padded by 8 each side) ----
            h1 = big.tile([128, DC, N + 16], bf16, tag="h1", bufs=1)
            nc.vector.memset(h1[:, :, 0:8], 0.0)
            nc.vector.memset(h1[:, :, N + 8:N + 16], 0.0)
            if "conv1" in skip:
                nc.vector.memset(h1[:, :, 8:N + 8], 0.1)
            for mc in range(0 if "conv1" in skip else DC):
                for c in range(LC):
                    pc = ps.tile([128, 512], f32, tag="ps")
                    for kc in range(DC):
                        nc.tensor.matmul(pc[:], w1[:, kc, mc, :],
                                         xth[:, 4 * c:4 * (c + 1), kc, :],
                                         start=(kc == 0), stop=(kc == DC - 1),
                                         skip_group_check=True)
                    nc.scalar.activation(h1[:, mc, 8 + c * 512:8 + (c + 1) * 512],
                                         pc[:], AF.Relu, bias=sh1[:, mc:mc + 1])

            # ---- conv2 (17 taps) + bn2 + residual + relu -> x_convT ----
            xcv = big.tile([128, DC, N], bf16, tag="xcv")
            if "conv2" in skip:
                nc.vector.memset(xcv[:], 0.0)
            for mc in range(0 if "conv2" in skip else DC):
                for c in range(LC):
                    pc2 = ps.tile([128, 512], f32, tag="ps")
                    first = True
                    for kc in range(DC):
                        for t in range(KT):
                            nc.tensor.matmul(
                                pc2[:], w2[:, kc, t, mc, :],
                                h1[:, kc, c * 512 + t:c * 512 + t + 512],
                                start=first, stop=(kc == DC - 1 and t == KT - 1),
                                skip_group_check=True)
                            first = False
                    tv = cvp.tile([128, 4, 128], f32, tag="cv", bufs=2)
                    nc.vector.tensor_tensor(tv[:], pc2[:].rearrange(
                        "p (a b) -> p a b", b=128),
                        xth[:, 4 * c:4 * (c + 1), mc, :], OP.add)
                    nc.vector.tensor_tensor(tv[:], tv[:],
                                            xtl[:, 4 * c:4 * (c + 1), mc, :],
                                            OP.add)
                    nc.scalar.activation(
                        xcv[:, mc, c * 512:(c + 1) * 512],
                        tv[:].rearrange("p a b -> p (a b)"),
                        AF.Relu, bias=sh2[:, mc:mc + 1])

            # ---- msg = relu(x @ w_msg.T + b_msg), n-major ----
            msg = big.tile([128, NB, D], bf16, tag="msg")
            if "msg" in skip:
                nc.vector.memset(msg[:], 0.01)
            for nb in range(0 if "msg" in skip else NB):
                pm = ps.tile([128, 512], f32, tag="ps")
                for kc in range(DC):
                    nc.tensor.matmul(pm[:, 0:D], xth[:, nb, kc, :],
                                     wmsg[:, kc, :], start=(kc == 0),
                                     stop=(not use_bias and kc == DC - 1),
                                     skip_group_check=True)
                if use_bias:
                    nc.tensor.matmul(pm[:, 0:D], ones[:], bmsg[:], start=False,
                                     stop=True, skip_group_check=True)
                nc.scalar.activation(msg[:, nb, :], pm[:, 0:D], AF.Relu)

            # ---- A-branch: x_nmpT[d, i] = sum_j exp(-dist[j,i]) * msg[j, d] ----
            xnm = big.tile([128, DC, N], bf16, tag="xnm")
            if "noG" in skip and el == 0:
                atc = wp.tile([128, 512], bf16, tag="atc")
                nc.vector.memset(atc[:], 0.001)
            if "A" in skip:
                nc.vector.memset(xnm[:], 0.0)

            # A-branch restructured into long clean matmul runs (conv2-style):
            # per i-chunk, batch all 16 G matmuls + exps, then run the 16
            # m0-accumulations back-to-back into one bank, then the 16 m1 —
            # software-pipelined one chunk deep (acc of chunk ic-1 overlaps
            # exp of chunk ic).  Interleaved short groups measured ~560 ns/MM
            # on HW vs ~240 ns/MM for long runs.
            at_store = {}

            def emit_acc(ic):
                accs = [pa.tile([128, 512], f32, tag="acc0", name="a0"),
                        pa.tile([128, 512], f32, tag="acc1", name="a1")]
                for mc in range(DC):
                    for jb in range(NB):
                        nc.tensor.matmul(accs[mc][:],
                                         msg[:, jb, mc * 128:(mc + 1) * 128],
                                         at_store[(ic, jb)][:],
                                         start=(jb == 0), stop=(jb == NB - 1),
                                         skip_group_check=True)
                for mc in range(DC):
                    nc.vector.tensor_copy(xnm[:, mc, ic * 512:(ic + 1) * 512],
                                          accs[mc][:])

            for ic in range(0 if "A" in skip else LC):
                for jb in range(NB):
                    if "noG" in skip:
                        at_store[(ic, jb)] = atc
                        continue
                    pgm = ps.tile([128, 512], f32, tag="ps")
                    nc.tensor.matmul(pgm[:], saT[0:13, jb, :],
                                     sbT[0:13, 4 * ic:4 * (ic + 1), :],
                                     start=True, stop=True,
                                     skip_group_check=True)
                    at = atp.tile([128, 512], bf16, tag="at")
                    if "exp2dve" in skip:
                        nc.vector.tensor_copy(at[:], pgm[:])
                    else:
                        nc.scalar.activation(at[:], pgm[:], AF.Exp)
                    if use_mask:
                        mtt = cvp.tile([128, 512], bf16, tag="mtt")
                        nc.sync.dma_start(mtt[:],
                                          mt_d[el, jb * 128:(jb + 1) * 128,
                                               ic * 512:(ic + 1) * 512])
                        nc.vector.tensor_tensor(at[:], at[:], mtt[:], OP.mult)
                    at_store[(ic, jb)] = at
                if ic >= 1:
                    emit_acc(ic - 1)
            if "A" not in skip:
                emit_acc(LC - 1)

            # ---- GRU gates (n-major) ----
            if "gru" in skip:
                for nb in range(NB):
                    sl = slice(nb * 128, (nb + 1) * 128)
                    nc.sync.dma_start(out_d[el, sl, :], xnf[:, nb, :])
            for nb in range(0 if "gru" in skip else NB):
                sl = slice(nb * 128, (nb + 1) * 128)
                prz = ps.tile([128, 512], f32, tag="ps")
                ih_srcs = [xcv[:, 0, sl], xcv[:, 1, sl], xnm[:, 0, sl], xnm[:, 1, sl]]
                hh_srcs = [xth[:, nb, 0, :], xth[:, nb, 1, :]]
                for ci in range(4):
                    nc.tensor.matmul(prz[:], ih_srcs[ci], wih[:, ci, 0:512],
                                     start=(ci == 0), stop=False,
                                     skip_group_check=True)
                for kc in range(DC):
                    nc.tensor.matmul(prz[:], hh_srcs[kc], whh[:, kc, 0:512],
                                     start=False,
                                     stop=(not use_bias and kc == DC - 1),
                                     skip_group_check=True)
                if use_bias:
                    nc.tensor.matmul(prz[:], ones[:], brz[:], start=False,
                                     stop=True, skip_group_check=True)

                # gi_n in cols 0:D, gh_n in cols D:2D of ONE psum tile —
                # 2 psum allocs per block instead of 3, so two blocks pipeline
                pgg = ps.tile([128, 512], f32, tag="ps")
                for ci in range(4):
                    nc.tensor.matmul(pgg[:, 0:D], ih_srcs[ci], wih[:, ci, 512:768],
                                     start=(ci == 0),
                                     stop=(not use_bias and ci == 3),
                                     skip_group_check=True)
                if use_bias:
                    nc.tensor.matmul(pgg[:, 0:D], ones[:], bgin[:], start=False,
                                     stop=True, skip_group_check=True)
                for kc in range(DC):
                    nc.tensor.matmul(pgg[:, D:2 * D], hh_srcs[kc],
                                     whh[:, kc, 512:768], start=(kc == 0),
                                     stop=(not use_bias and kc == DC - 1),
                                     skip_group_check=True)
                if use_bias:
                    nc.tensor.matmul(pgg[:, D:2 * D], ones[:], bghn[:],
                                     start=False, stop=True,
                                     skip_group_check=True)

                # w_hh n-gate columns are pre-scaled by 0.5 on the host, so
                # pgg[:, D:2D] = 0.5*gh_n =: g.  With tr = tanh(0.5*(r-gate)):
                #   r*gh_n = ((tr+1)/2)*gh_n = (tr+1)*g
                #   out    = nn + z*(x-nn) = nn + 0.5*(tz+1)*(x-nn)
                tr = gtp.tile([128, D], f32, tag="tr", bufs=3)
                nc.scalar.activation(tr[:], prz[:, 0:D], AF.Tanh, scale=0.5)
                tz = gtp.tile([128, D], f32, tag="tz", bufs=3)
                nc.scalar.activation(tz[:], prz[:, D:2 * D], AF.Tanh, scale=0.5)
                q = gtp.tile([128, D], f32, tag="q", bufs=3)
                nc.vector.scalar_tensor_tensor(q[:], tr[:], 1.0, pgg[:, D:2 * D],
                                               OP.add, OP.mult)
                nc.vector.tensor_tensor(q[:], q[:], pgg[:, 0:D], OP.add)
                nn = gtp.tile([128, D], f32, tag="nn", bufs=3)
                nc.scalar.activation(nn[:], q[:], AF.Tanh)
                dd = gtp.tile([128, D], f32, tag="dd", bufs=3)
                nc.vector.tensor_tensor(dd[:], xnf[:, nb, :], nn[:], OP.subtract)
                nc.vector.scalar_tensor_tensor(dd[:], tz[:], 1.0, dd[:],
                                               OP.add, OP.mult)
                ho = gtp.tile([128, D], f32, tag="ho", bufs=3)
                nc.vector.scalar_tensor_tensor(ho[:], dd[:], 0.5, nn[:],
                                               OP.mult, OP.add)
                nc.sync.dma_start(out_d[el, sl, :], ho[:])

    nc.compile()
    return nc


def _host_prep(inputs):
    g = {k: np.asarray(v, np.float32) for k, v in inputs.items()}
    sc1 = g["bn1_g"] / np.sqrt(g["bn1_v"] + EPS)
    sh1 = g["bn1_b"] - g["bn1_m"] * sc1
    sc2 = g["bn2_g"] / np.sqrt(g["bn2_v"] + EPS)
    sh2 = g["bn2_b"] - g["bn2_m"] * sc2

    w1p = g["conv1_w"][:, :, 0] * sc1[:, None]          # (O, I)
    w2p = g["conv2_w"] * sc2[:, None, None]             # (O, I, 17)

    def lhsT_pack(w):   # (O, I) -> (128, kc=I/128, mc=O/128, 128): [p,kc,mc,m]
        o, i = w.shape
        return np.ascontiguousarray(np.transpose(
            w.T.reshape(i // 128, 128, o // 128, 128), (1, 0, 2, 3)))

    w1t = lhsT_pack(w1p).astype(BF)
    w2t = np.stack([lhsT_pack(w2p[:, :, t]) for t in range(KT)], axis=2)
    w2t = np.ascontiguousarray(np.transpose(w2t, (0, 1, 2, 3, 4)))  # [p,kc,t,mc,m]
    w2t = w2t.astype(BF)

    def rhs_pack(wt):   # (Kdim, F) -> (128, kc, F)
        k, f = wt.shape
        return np.ascontiguousarray(
            np.transpose(wt.reshape(k // 128, 128, f), (1, 0, 2)))

    wmsgt = rhs_pack(g["w_msg"].T).astype(BF)
    wiht = rhs_pack(g["w_ih"].T).astype(BF)
    whh_t = g["w_hh"].T.copy()                           # (256, 768)
    whh_t[:, 2 * D:] *= 0.5                              # n-gate pre-scaled
    whht = rhs_pack(whh_t).astype(BF)

    wse_t = g["w_se"].T                                  # (256, 3)
    wse_hi = wse_t.astype(BF)
    wse_lo = (wse_t - wse_hi.astype(np.float32)).astype(BF)
    wsehl = rhs_pack(np.concatenate(
        [wse_hi.astype(np.float32), wse_lo.astype(np.float32)],
        axis=1)).astype(BF)

    bih, bhh = g["b_ih"], g["b_hh"]
    feed = {
        "w1t": w1t, "w2t": w2t,
        "sh1": np.ascontiguousarray(sh1.reshape(DC, 128).T.astype(np.float32)),
        "sh2": np.ascontiguousarray(sh2.reshape(DC, 128).T.astype(np.float32)),
        "wmsgt": wmsgt, "bmsg": g["b_msg"].reshape(1, D).astype(BF),
        "wsehl": wsehl,
        "bse": g["b_se"].reshape(1, 3).astype(BF),
        "wiht": wiht, "whht": whht,
        "brow_rz": (bih[:2 * D] + bhh[:2 * D]).reshape(1, 2 * D).astype(BF),
        "brow_gin": bih[2 * D:].reshape(1, D).astype(BF),
        "brow_ghn": (0.5 * bhh[2 * D:]).reshape(1, D).astype(BF),
        "ones128": np.ones((1, 128), BF),
        "ones512": np.ones((1, 512), BF),
        "ones2n": np.ones((2, N), BF),
    }
    return g, feed


def make_in_maps(inputs):
    g, feed = _host_prep(inputs)
    x = g["x"]
    mask = g["mask"]
    use_mask = not bool(np.all(mask == 1.0))
    use_bias = not (np.all(g["b_se"] == 0) and np.all(g["b_msg"] == 0)
                    and np.all(g["b_ih"] == 0) and np.all(g["b_hh"] == 0))
    in_maps = []
    for i in range(NCORE):
        m = dict(feed)
        m["x"] = np.ascontiguousarray(x[i * PER:(i + 1) * PER])
        if use_mask:
            m["maskt"] = np.ascontiguousarray(
                mask[i * PER:(i + 1) * PER].transpose(0, 2, 1)).astype(BF)
        in_maps.append(m)
    return in_maps, use_mask, use_bias


def get_nc(use_mask: bool, use_bias: bool = True):
    key = (use_mask, use_bias)
    if key not in _built:
        _built[key] = _build(use_mask, use_bias)
    return _built[key]


def kernel(**inputs) -> np.ndarray:
    in_maps, use_mask, use_bias = make_in_maps(inputs)
    nc = get_nc(use_mask, use_bias)
    from concourse import bass_utils
    last_err = None
    for attempt in range(3):
        try:
            res = bass_utils.run_bass_kernel_spmd(nc, in_maps,
                                                  core_ids=list(range(NCORE)))
            out = np.concatenate([res.results[i]["out"] for i in range(NCORE)],
                                 axis=0)
            return np.ascontiguousarray(out.astype(np.float32))
        except Exception as e:  # wedged device: reset backend and retry
            last_err = e
            try:
                import jax
                jax.clear_caches()
                jax.extend.backend.clear_backends()
            except Exception:
                pass
            import time as _t
            _t.sleep(5)
    raise last_err



# revision 21
# speedup vs baseline: 1.4798x; 1.0161x over previous
"""Trainium2 Bass kernel for nn_ConvolutionalNMPBlock.

Self-contained: takes full (unsharded) inputs, shards batch across 8
NeuronCores (2 elements each), runs a fused Bass/Tile kernel, gathers.
"""
import numpy as np
import ml_dtypes

BS, N, D = 16, 2048, 256
NCORE = 8
PER = BS // NCORE          # batch elements per core
EPS = 1e-5
NB = N // 128              # 16 row blocks
LC = N // 512              # 4 column chunks of 512
DC = D // 128              # 2 channel blocks
KT = 17                    # conv2 taps
BF = ml_dtypes.bfloat16

_built = {}                # use_mask -> compiled nc


def _build(use_mask: bool, use_bias: bool = True, loop_n: int = 1,
           skip: frozenset = frozenset(), body_reps: int = 1):
    from concourse import bacc, tile
    import concourse.mybir as mybir
    from contextlib import ExitStack

    f32 = mybir.dt.float32
    bf16 = mybir.dt.bfloat16
    AF = mybir.ActivationFunctionType
    OP = mybir.AluOpType

    nc = bacc.Bacc("TRN2", target_bir_lowering=False, debug=False,
                   num_devices=NCORE)

    def din(name, shape, dt=f32):
        return nc.dram_tensor(name, shape, dt, kind="ExternalInput").ap()

    x_d = din("x", (PER, N, D))
    w1_d = din("w1t", (128, DC, DC, 128), bf16)        # [p=kin, kc, mc, m]
    w2_d = din("w2t", (128, DC, KT, DC, 128), bf16)    # [p, kc, tap, mc, m]
    sh1_d = din("sh1", (128, DC))
    sh2_d = din("sh2", (128, DC))
    wmsg_d = din("wmsgt", (128, DC, D), bf16)          # [p, kc, f]
    bmsg_d = din("bmsg", (1, D), bf16)
    wsehl_d = din("wsehl", (128, DC, 6), bf16)         # [p, kc, hi(3)|lo(3)]
    bse_d = din("bse", (1, 3), bf16)
    wih_d = din("wiht", (128, 4, 3 * D), bf16)         # [p, kc, f]
    whh_d = din("whht", (128, DC, 3 * D), bf16)
    brz_d = din("brow_rz", (1, 2 * D), bf16)
    bgin_d = din("brow_gin", (1, D), bf16)
    bghn_d = din("brow_ghn", (1, D), bf16)
    ones_d = din("ones128", (1, 128), bf16)
    ones5_d = din("ones512", (1, 512), bf16)
    ones2n_d = din("ones2n", (2, N), bf16)
    if use_mask:
        mt_d = din("maskt", (PER, N, N), bf16)
    out_d = nc.dram_tensor("out", (PER, N, D), f32, kind="ExternalOutput").ap()

    with tile.TileContext(nc) as tc, ExitStack() as ctx:
        if loop_n > 1:
            ctx.enter_context(tc.For_i(0, loop_n, 1))
        wp = ctx.enter_context(tc.tile_pool(name="wp", bufs=1))
        big = ctx.enter_context(tc.tile_pool(name="big", bufs=1))
        rawp = ctx.enter_context(tc.tile_pool(name="rawp", bufs=1))
        atp = ctx.enter_context(tc.tile_pool(name="atp", bufs=33))
        cvp = ctx.enter_context(tc.tile_pool(name="cvp", bufs=2))
        gtp = ctx.enter_context(tc.tile_pool(name="gtp", bufs=1))
        dramp = ctx.enter_context(tc.tile_pool(name="dramp", bufs=1, space="DRAM"))
        ps = ctx.enter_context(tc.tile_pool(name="ps", bufs=4, space="PSUM"))
        pa = ctx.enter_context(tc.tile_pool(name="pa", bufs=2, space="PSUM"))

        # ---- load weights (persistent) ----
        w1 = wp.tile([128, DC, DC, 128], bf16, tag="w1")
        nc.gpsimd.dma_start(w1[:], w1_d[:])
        sh1 = wp.tile([128, DC], f32, tag="sh1")
        nc.gpsimd.dma_start(sh1[:], sh1_d[:])
        sh2 = wp.tile([128, DC], f32, tag="sh2")
        nc.gpsimd.dma_start(sh2[:], sh2_d[:])
        bmsg = wp.tile([1, D], bf16, tag="bmsg")
        nc.gpsimd.dma_start(bmsg[:], bmsg_d[:])
        wsehl = wp.tile([128, DC, 6], bf16, tag="wsehl")
        nc.gpsimd.dma_start(wsehl[:], wsehl_d[:])
        bse = wp.tile([1, 3], bf16, tag="bse")
        nc.gpsimd.dma_start(bse[:], bse_d[:])
        ones = wp.tile([1, 128], bf16, tag="ones")
        nc.gpsimd.dma_start(ones[:], ones_d[:])
        ones5 = wp.tile([1, 512], bf16, tag="ones5")
        nc.gpsimd.dma_start(ones5[:], ones5_d[:])
        # n-major sources for the 13-row RBF factors; rows live in cols 0:13,
        # cols 13:128 zeroed once here (transposed junk would land in unused
        # partitions anyway, but keep the sim's finite-checks happy).
        saSRC = wp.tile([128, NB, 128], bf16, tag="saSRC")
        nc.vector.memset(saSRC[:], 0.0)
        nc.vector.memset(saSRC[:, :, 9:11], 1.0)
        sbSRC = wp.tile([128, NB, 128], bf16, tag="sbSRC")
        nc.vector.memset(sbSRC[:], 0.0)
        nc.vector.memset(sbSRC[:, :, 11:13], 1.0)

        def emit_load(el):
            # ---- load x n-major (contiguous), split bf16 hi/lo, and build
            # channel-major copies via the XBAR transpose DMA (2-byte dtype).
            xnf = rawp.tile([128, NB, D], f32, tag="xnf", bufs=2)
            xnh = rawp.tile([128, NB, D], bf16, tag="xnh", bufs=1)
            xnl = rawp.tile([128, NB, D], bf16, tag="xnl", bufs=1)
            # layout: xth[p_c, nb, dc, p_n] == xT[dc*128+p_c, nb*128+p_n]
            xth = big.tile([128, NB, DC, 128], bf16, tag="xth", bufs=2)
            xtl = big.tile([128, NB, DC, 128], bf16, tag="xtl", bufs=2)
            qn = NB // 4
            quarters = [slice(h * qn, (h + 1) * qn) for h in range(4)]
            for h_, hs in enumerate(quarters):
                nc.sync.dma_start(
                    xnf[:, hs, :],
                    x_d[el, h_ * (N // 4):(h_ + 1) * (N // 4), :].rearrange(
                        "(nb p) d -> p nb d", p=128))
            for hs in quarters:
                nc.vector.tensor_copy(xnh[:, hs, :], xnf[:, hs, :])
                nc.vector.scalar_tensor_tensor(xnl[:, hs, :], xnf[:, hs, :],
                                               1.0, xnh[:, hs, :],
                                               OP.mult, OP.subtract)
            # transposes on the sync HWDGE queue — on the scalar (Act) queue
            # they'd serialize behind the previous element's exp/tanh stream.
            for hs in quarters:
                nc.sync.dma_start_transpose(
                    xth[:, hs].rearrange("p nb dc pn -> p (nb dc) pn"),
                    xnh[:, hs, :])
            for hs in quarters:
                nc.sync.dma_start_transpose(
                    xtl[:, hs].rearrange("p nb dc pn -> p (nb dc) pn"),
                    xnl[:, hs, :])
            return xnf, xth, xtl

        els = [e for _ in range(body_reps) for e in range(PER)]
        pending = {0: emit_load(els[0])}
        for idx, el in enumerate(els):
            xnf, xth, xtl = pending.pop(idx)

            if el == 0:
                w2 = wp.tile([128, DC, KT, DC, 128], bf16, tag="w2")
                nc.sync.dma_start(w2[:], w2_d[:])
                wmsg = wp.tile([128, DC, D], bf16, tag="wmsg")
                nc.sync.dma_start(wmsg[:], wmsg_d[:])
                wih = wp.tile([128, 4, 3 * D], bf16, tag="wih")
                nc.sync.dma_start(wih[:], wih_d[:])
                whh = wp.tile([128, DC, 3 * D], bf16, tag="whh")
                nc.sync.dma_start(whh[:], whh_d[:])
                brz = wp.tile([1, 2 * D], bf16, tag="brz")
                nc.sync.dma_start(brz[:], brz_d[:])
                bgin = wp.tile([1, D], bf16, tag="bgin")
                nc.sync.dma_start(bgin[:], bgin_d[:])
                bghn = wp.tile([1, D], bf16, tag="bghn")
                nc.sync.dma_start(bghn[:], bghn_d[:])
            # ---- s = x @ w_se.T + b_se computed n-major (f32 via hi/lo),
            # 13-row RBF factors assembled wide (128 lanes, free-dim slices
            # only — compute engines can't address odd partition bases), then
            # XBAR-transposed to the [13 rows, n] layout the G matmul needs.
            # Row pairs (SA | SB): 0:3 (2s_hi | s_hi), 3:6 (2s_lo | s_hi),
            # 6:9 (2s_hi | s_lo), 9,10 (1 | -sq_hi, -sq_lo),
            # 11,12 (-sq_hi, -sq_lo | 1).
            saT = big.tile([128, NB, 128], bf16, tag="saT", bufs=1)
            sbT = big.tile([128, NB, 128], bf16, tag="sbT", bufs=1)
            if "se" in skip:
                nc.vector.memset(saT[:], 0.001)
                nc.vector.memset(sbT[:], 0.001)
            else:
                pss = ps.tile([128, 512], f32, tag="ps")
                for nb in range(NB):
                    c6 = slice(6 * nb, 6 * nb + 6)
                    c3 = slice(6 * nb, 6 * nb + 3)
                    nc.tensor.matmul(pss[:, c6], xth[:, nb, 0, :],
                                     wsehl[:, 0, :], start=True, stop=False,
                                     skip_group_check=True)
                    nc.tensor.matmul(pss[:, c6], xth[:, nb, 1, :],
                                     wsehl[:, 1, :], start=False, stop=False,
                                     skip_group_check=True)
                    nc.tensor.matmul(pss[:, c3], xtl[:, nb, 0, :],
                                     wsehl[:, 0, 0:3], start=False, stop=False,
                                     skip_group_check=True)
                    nc.tensor.matmul(pss[:, c3], xtl[:, nb, 1, :],
                                     wsehl[:, 1, 0:3], start=False,
                                     stop=not use_bias, skip_group_check=True)
                    if use_bias:
                        nc.tensor.matmul(pss[:, c3], ones[:], bse[:],
                                         start=False, stop=True,
                                         skip_group_check=True)
                pssv = pss[:, 0:6 * NB].rearrange("p (nb c) -> p nb c", c=6)
                snm = cvp.tile([128, NB, 3], f32, tag="snm", bufs=1)
                nc.vector.tensor_copy(snm[:], pssv[:, :, 3:6])
                nc.vector.tensor_tensor(snm[:], snm[:], pssv[:, :, 0:3], OP.add)
                shi = cvp.tile([128, NB, 3], bf16, tag="shi", bufs=1)
                nc.vector.tensor_copy(shi[:], snm[:])
                slo = cvp.tile([128, NB, 3], bf16, tag="slo", bufs=1)
                nc.vector.scalar_tensor_tensor(slo[:], snm[:], 1.0, shi[:],
                                               OP.mult, OP.subtract)
                ssq = cvp.tile([128, NB, 3], f32, tag="ssq", bufs=1)
                nc.scalar.activation(ssq[:], snm[:], AF.Square)
                sqn = cvp.tile([128, NB, 1], f32, tag="sqn", bufs=1)
                nc.vector.tensor_tensor(sqn[:], ssq[:, :, 0:1], ssq[:, :, 1:2],
                                        OP.add)
                nc.vector.tensor_tensor(sqn[:], sqn[:], ssq[:, :, 2:3], OP.add)
                nc.vector.tensor_scalar(saSRC[:, :, 0:3], shi[:], 2.0, None,
                                        OP.mult)
                nc.vector.tensor_scalar(saSRC[:, :, 3:6], slo[:], 2.0, None,
                                        OP.mult)
                nc.vector.tensor_copy(saSRC[:, :, 6:9], saSRC[:, :, 0:3])
                nc.vector.tensor_scalar(saSRC[:, :, 11:12], sqn[:], -1.0, None,
                                        OP.mult)
                nc.vector.scalar_tensor_tensor(saSRC[:, :, 12:13], sqn[:], -1.0,
                                               saSRC[:, :, 11:12],
                                               OP.mult, OP.subtract)
                nc.vector.tensor_copy(sbSRC[:, :, 0:3], shi[:])
                nc.vector.tensor_copy(sbSRC[:, :, 3:6], shi[:])
                nc.vector.tensor_copy(sbSRC[:, :, 6:9], slo[:])
                nc.vector.tensor_copy(sbSRC[:, :, 9:11], saSRC[:, :, 11:13])
                nc.sync.dma_start_transpose(
                    saT[:], saSRC[:].rearrange("p nb c -> p (nb c)"))
                nc.sync.dma_start_transpose(
                    sbT[:], sbSRC[:].rearrange("p nb c -> p (nb c)"))

            # ---- conv1 (1x1) + bn1 + relu -> h1 (padded by 8 each side) ----
            h1 = big.tile([128, DC, N + 16], bf16, tag="h1", bufs=1)
            nc.vector.memset(h1[:, :, 0:8], 0.0)
            nc.vector.memset(h1[:, :, N + 8:N + 16], 0.0)
            if "conv1" in skip:
                nc.vector.memset(h1[:, :, 8:N + 8], 0.1)
            for mc in range(0 if "conv1" in skip else DC):
                for c in range(LC):
                    pc = ps.tile([128, 512], f32, tag="ps")
                    for kc in range(DC):
                        nc.tensor.matmul(pc[:], w1[:, kc, mc, :],
                                         xth[:, 4 * c:4 * (c + 1), kc, :],
                                         start=(kc == 0), stop=(kc == DC - 1),
                                         skip_group_check=True)
                    nc.scalar.activation(h1[:, mc, 8 + c * 512:8 + (c + 1) * 512],
                                         pc[:], AF.Relu, bias=sh1[:, mc:mc + 1])

            # prefetch next element's load/split/transposes here: DVE and the
            # sync queue are near-idle during conv2, and it keeps the PE
            # stream dense across the element boundary (HAM stays warm).
            if idx + 1 < len(els):
                pending[idx + 1] = emit_load(els[idx + 1])

            # ---- conv2 (17 taps) + bn2 + residual + relu -> x_convT ----
            xcv = big.tile([128, DC, N], bf16, tag="xcv")
            if "conv2" in skip:
                nc.vector.memset(xcv[:], 0.0)
            for mc in range(0 if "conv2" in skip else DC):
                for c in range(LC):
                    pc2 = ps.tile([128, 512], f32, tag="ps")
                    first = True
                    for kc in range(DC):
                        for t in range(KT):
                            nc.tensor.matmul(
                                pc2[:], w2[:, kc, t, mc, :],
                                h1[:, kc, c * 512 + t:c * 512 + t + 512],
                                start=first, stop=(kc == DC - 1 and t == KT - 1),
                                skip_group_check=True)
                            first = False
                    tv = cvp.tile([128, 4, 128], f32, tag="cv", bufs=2)
                    nc.vector.tensor_tensor(tv[:], pc2[:].rearrange(
                        "p (a b) -> p a b", b=128),
                        xth[:, 4 * c:4 * (c + 1), mc, :], OP.add)
                    nc.vector.tensor_tensor(tv[:], tv[:],
                                            xtl[:, 4 * c:4 * (c + 1), mc, :],
                                            OP.add)
                    nc.scalar.activation(
                        xcv[:, mc, c * 512:(c + 1) * 512],
                        tv[:].rearrange("p a b -> p (a b)"),
                        AF.Relu, bias=sh2[:, mc:mc + 1])

            # ---- msg = relu(x @ w_msg.T + b_msg), n-major ----
            msg = big.tile([128, NB, D], bf16, tag="msg")
            if "msg" in skip:
                nc.vector.memset(msg[:], 0.01)
            for nb in range(0 if "msg" in skip else NB):
                pm = ps.tile([128, 512], f32, tag="ps")
                for kc in range(DC):
                    nc.tensor.matmul(pm[:, 0:D], xth[:, nb, kc, :],
                                     wmsg[:, kc, :], start=(kc == 0),
                                     stop=(not use_bias and kc == DC - 1),
                                     skip_group_check=True)
                if use_bias:
                    nc.tensor.matmul(pm[:, 0:D], ones[:], bmsg[:], start=False,
                                     stop=True, skip_group_check=True)
                nc.scalar.activation(msg[:, nb, :], pm[:, 0:D], AF.Relu)

            # ---- A-branch: x_nmpT[d, i] = sum_j exp(-dist[j,i]) * msg[j, d] ----
            xnm = big.tile([128, DC, N], bf16, tag="xnm")
            if "noG" in skip and el == 0:
                atc = wp.tile([128, 512], bf16, tag="atc")
                nc.vector.memset(atc[:], 0.001)
            if "A" in skip:
                nc.vector.memset(xnm[:], 0.0)

            # A-branch restructured into long clean matmul runs (conv2-style):
            # per i-chunk, batch all 16 G matmuls + exps, then run the 16
            # m0-accumulations back-to-back into one bank, then the 16 m1 —
            # software-pipelined one chunk deep (acc of chunk ic-1 overlaps
            # exp of chunk ic).  Interleaved short groups measured ~560 ns/MM
            # on HW vs ~240 ns/MM for long runs.
            at_store = {}

            def emit_acc(ic):
                accs = [pa.tile([128, 512], f32, tag="acc0", name="a0"),
                        pa.tile([128, 512], f32, tag="acc1", name="a1")]
                for mc in range(DC):
                    for jb in range(NB):
                        nc.tensor.matmul(accs[mc][:],
                                         msg[:, jb, mc * 128:(mc + 1) * 128],
                                         at_store[(ic, jb)][:],
                                         start=(jb == 0), stop=(jb == NB - 1),
                                         skip_group_check=True)
                for mc in range(DC):
                    nc.vector.tensor_copy(xnm[:, mc, ic * 512:(ic + 1) * 512],
                                          accs[mc][:])

            for ic in range(0 if "A" in skip else LC):
                for jb in range(NB):
                    if "noG" in skip:
                        at_store[(ic, jb)] = atc
                        continue
                    pgm = ps.tile([128, 512], f32, tag="ps")
                    nc.tensor.matmul(pgm[:], saT[0:13, jb, :],
                                     sbT[0:13, 4 * ic:4 * (ic + 1), :],
                                     start=True, stop=True,
                                     skip_group_check=True)
                    at = atp.tile([128, 512], bf16, tag="at")
                    if "exp2dve" in skip:
                        nc.vector.tensor_copy(at[:], pgm[:])
                    else:
                        nc.scalar.activation(at[:], pgm[:], AF.Exp)
                    if use_mask:
                        mtt = cvp.tile([128, 512], bf16, tag="mtt")
                        nc.sync.dma_start(mtt[:],
                                          mt_d[el, jb * 128:(jb + 1) * 128,
                                               ic * 512:(ic + 1) * 512])
                        nc.vector.tensor_tensor(at[:], at[:], mtt[:], OP.mult)
                    at_store[(ic, jb)] = at
                if ic >= 1:
                    emit_acc(ic - 1)
            if "A" not in skip:
                emit_acc(LC - 1)

            # ---- GRU gates (n-major) ----
            if "gru" in skip:
                for nb in range(NB):
                    sl = slice(nb * 128, (nb + 1) * 128)
                    nc.sync.dma_start(out_d[el, sl, :], xnf[:, nb, :])
            for nb in range(0 if "gru" in skip else NB):
                sl = slice(nb * 128, (nb + 1) * 128)
                prz = ps.tile([128, 512], f32, tag="ps")
                ih_srcs = [xcv[:, 0, sl], xcv[:, 1, sl], xnm[:, 0, sl], xnm[:, 1, sl]]
                hh_srcs = [xth[:, nb, 0, :], xth[:, nb, 1, :]]
                for ci in range(4):
                    nc.tensor.matmul(prz[:], ih_srcs[ci], wih[:, ci, 0:512],
                                     start=(ci == 0), stop=False,
                                     skip_group_check=True)
                for kc in range(DC):
                    nc.tensor.matmul(prz[:], hh_srcs[kc], whh[:, kc, 0:512],
                                     start=False,
                                     stop=(not use_bias and kc == DC - 1),
                                     skip_group_check=True)
                if use_bias:
                    nc.tensor.matmul(prz[:], ones[:], brz[:], start=False,
                                     stop=True, skip_group_check=True)

                # gi_n in cols 0:D, gh_n in cols D:2D of ONE psum tile —
                # 2 psum allocs per block instead of 3, so two blocks pipeline
                pgg = ps.tile([128, 512], f32, tag="ps")
                for ci in range(4):
                    nc.tensor.matmul(pgg[:, 0:D], ih_srcs[ci], wih[:, ci, 512:768],
                                     start=(ci == 0),
                                     stop=(not use_bias and ci == 3),
                                     skip_group_check=True)
                if use_bias:
                    nc.tensor.matmul(pgg[:, 0:D], ones[:], bgin[:], start=False,
                                     stop=True, skip_group_check=True)
                for kc in range(DC):
                    nc.tensor.matmul(pgg[:, D:2 * D], hh_srcs[kc],
                                     whh[:, kc, 512:768], start=(kc == 0),
                                     stop=(not use_bias and kc == DC - 1),
                                     skip_group_check=True)
                if use_bias:
                    nc.tensor.matmul(pgg[:, D:2 * D], ones[:], bghn[:],
                                     start=False, stop=True,
                                     skip_group_check=True)

                # w_hh n-gate columns are pre-scaled by 0.5 on the host, so
                # pgg[:, D:2D] = 0.5*gh_n =: g.  With tr = tanh(0.5*(r-gate)):
                #   r*gh_n = ((tr+1)/2)*gh_n = (tr+1)*g
                #   out    = nn + z*(x-nn) = nn + 0.5*(tz+1)*(x-nn)
                tr = gtp.tile([128, D], f32, tag="tr", bufs=2)
                nc.scalar.activation(tr[:], prz[:, 0:D], AF.Tanh, scale=0.5)
                tz = gtp.tile([128, D], f32, tag="tz", bufs=2)
                nc.scalar.activation(tz[:], prz[:, D:2 * D], AF.Tanh, scale=0.5)
                q = gtp.tile([128, D], f32, tag="q", bufs=1)
                nc.vector.scalar_tensor_tensor(q[:], tr[:], 1.0, pgg[:, D:2 * D],
                                               OP.add, OP.mult)
                nc.vector.tensor_tensor(q[:], q[:], pgg[:, 0:D], OP.add)
                nn = gtp.tile([128, D], f32, tag="nn", bufs=2)
                nc.scalar.activation(nn[:], q[:], AF.Tanh)
                dd = gtp.tile([128, D], f32, tag="dd", bufs=1)
                nc.vector.tensor_tensor(dd[:], xnf[:, nb, :], nn[:], OP.subtract)
                nc.vector.scalar_tensor_tensor(dd[:], tz[:], 1.0, dd[:],
                                               OP.add, OP.mult)
                ho = gtp.tile([128, D], f32, tag="ho", bufs=2)
                nc.vector.scalar_tensor_tensor(ho[:], dd[:], 0.5, nn[:],
                                               OP.mult, OP.add)
                nc.sync.dma_start(out_d[el, sl, :], ho[:])

    nc.compile()
    return nc


def _host_prep(inputs):
    g = {k: np.asarray(v, np.float32) for k, v in inputs.items()}
    sc1 = g["bn1_g"] / np.sqrt(g["bn1_v"] + EPS)
    sh1 = g["bn1_b"] - g["bn1_m"] * sc1
    sc2 = g["bn2_g"] / np.sqrt(g["bn2_v"] + EPS)
    sh2 = g["bn2_b"] - g["bn2_m"] * sc2

    w1p = g["conv1_w"][:, :, 0] * sc1[:, None]          # (O, I)
    w2p = g["conv2_w"] * sc2[:, None, None]             # (O, I, 17)

    def lhsT_pack(w):   # (O, I) -> (128, kc=I/128, mc=O/128, 128): [p,kc,mc,m]
        o, i = w.shape
        return np.ascontiguousarray(np.transpose(
            w.T.reshape(i // 128, 128, o // 128, 128), (1, 0, 2, 3)))

    w1t = lhsT_pack(w1p).astype(BF)
    w2t = np.stack([lhsT_pack(w2p[:, :, t]) for t in range(KT)], axis=2)
    w2t = np.ascontiguousarray(np.transpose(w2t, (0, 1, 2, 3, 4)))  # [p,kc,t,mc,m]
    w2t = w2t.astype(BF)

    def rhs_pack(wt):   # (Kdim, F) -> (128, kc, F)
        k, f = wt.shape
        return np.ascontiguousarray(
            np.transpose(wt.reshape(k // 128, 128, f), (1, 0, 2)))

    wmsgt = rhs_pack(g["w_msg"].T).astype(BF)
    wiht = rhs_pack(g["w_ih"].T).astype(BF)
    whh_t = g["w_hh"].T.copy()                           # (256, 768)
    whh_t[:, 2 * D:] *= 0.5                              # n-gate pre-scaled
    whht = rhs_pack(whh_t).astype(BF)

    wse_t = g["w_se"].T                                  # (256, 3)
    wse_hi = wse_t.astype(BF)
    wse_lo = (wse_t - wse_hi.astype(np.float32)).astype(BF)
    wsehl = rhs_pack(np.concatenate(
        [wse_hi.astype(np.float32), wse_lo.astype(np.float32)],
        axis=1)).astype(BF)

    bih, bhh = g["b_ih"], g["b_hh"]
    feed = {
        "w1t": w1t, "w2t": w2t,
        "sh1": np.ascontiguousarray(sh1.reshape(DC, 128).T.astype(np.float32)),
        "sh2": np.ascontiguousarray(sh2.reshape(DC, 128).T.astype(np.float32)),
        "wmsgt": wmsgt, "bmsg": g["b_msg"].reshape(1, D).astype(BF),
        "wsehl": wsehl,
        "bse": g["b_se"].reshape(1, 3).astype(BF),
        "wiht": wiht, "whht": whht,
        "brow_rz": (bih[:2 * D] + bhh[:2 * D]).reshape(1, 2 * D).astype(BF),
        "brow_gin": bih[2 * D:].reshape(1, D).astype(BF),
        "brow_ghn": (0.5 * bhh[2 * D:]).reshape(1, D).astype(BF),
        "ones128": np.ones((1, 128), BF),
        "ones512": np.ones((1, 512), BF),
        "ones2n": np.ones((2, N), BF),
    }
    return g, feed


def make_in_maps(inputs):
    g, feed = _host_prep(inputs)
    x = g["x"]
    mask = g["mask"]
    use_mask = not bool(np.all(mask == 1.0))
    use_bias = not (np.all(g["b_se"] == 0) and np.all(g["b_msg"] == 0)
                    and np.all(g["b_ih"] == 0) and np.all(g["b_hh"] == 0))
    in_maps = []
    for i in range(NCORE):
        m = dict(feed)
        m["x"] = np.ascontiguousarray(x[i * PER:(i + 1) * PER])
        if use_mask:
            m["maskt"] = np.ascontiguousarray(
                mask[i * PER:(i + 1) * PER].transpose(0, 2, 1)).astype(BF)
        in_maps.append(m)
    return in_maps, use_mask, use_bias


def get_nc(use_mask: bool, use_bias: bool = True):
    key = (use_mask, use_bias)
    if key not in _built:
        _built[key] = _build(use_mask, use_bias)
    return _built[key]


def kernel(**inputs) -> np.ndarray:
    in_maps, use_mask, use_bias = make_in_maps(inputs)
    nc = get_nc(use_mask, use_bias)
    from concourse import bass_utils
    last_err = None
    for attempt in range(3):
        try:
            res = bass_utils.run_bass_kernel_spmd(nc, in_maps,
                                                  core_ids=list(range(NCORE)))
            out = np.concatenate([res.results[i]["out"] for i in range(NCORE)],
                                 axis=0)
            return np.ascontiguousarray(out.astype(np.float32))
        except Exception as e:  # wedged device: reset backend and retry
            last_err = e
            try:
                import jax
                jax.clear_caches()
                jax.extend.backend.clear_backends()
            except Exception:
                pass
            import time as _t
            _t.sleep(5)
    raise last_err



# revision 27
# speedup vs baseline: 1.8681x; 1.2624x over previous
"""Trainium2 Bass kernel for nn_ConvolutionalNMPBlock.

Self-contained: takes full (unsharded) inputs, shards batch across 8
NeuronCores (2 elements each), runs a fused Bass/Tile kernel, gathers.
"""
import numpy as np
import ml_dtypes

BS, N, D = 16, 2048, 256
NCORE = 8
PER = BS // NCORE          # batch elements per core
EPS = 1e-5
NB = N // 128              # 16 row blocks
LC = N // 512              # 4 column chunks of 512
DC = D // 128              # 2 channel blocks
KT = 17                    # conv2 taps
BF = ml_dtypes.bfloat16
E4 = ml_dtypes.float8_e4m3

_built = {}                # use_mask -> compiled nc


def _build(use_mask: bool, use_bias: bool = True, loop_n: int = 1,
           skip: frozenset = frozenset(), body_reps: int = 1):
    from concourse import bacc, tile
    import concourse.mybir as mybir
    from contextlib import ExitStack

    f32 = mybir.dt.float32
    bf16 = mybir.dt.bfloat16
    AF = mybir.ActivationFunctionType
    OP = mybir.AluOpType

    nc = bacc.Bacc("TRN2", target_bir_lowering=False, debug=False,
                   num_devices=NCORE)

    def din(name, shape, dt=f32):
        return nc.dram_tensor(name, shape, dt, kind="ExternalInput").ap()

    fp8 = mybir.dt.float8e4
    x_d = din("x", (PER, N, D))
    w1_d = din("w1t", (128, DC, DC, 128), bf16)        # [p=kin, kc, mc, m]
    w2_d = din("w2t", (128, DC, KT, DC, 128), fp8)     # [p, kc, tap, mc, m]
    sh1_d = din("sh1", (128, DC))
    sh2_d = din("sh2", (128, DC))
    wmsg_d = din("wmsgt", (128, DC, D), bf16)          # [p, kc, f]
    bmsg_d = din("bmsg", (1, D), bf16)
    wsehl_d = din("wsehl", (128, DC, 6), bf16)         # [p, kc, hi(3)|lo(3)]
    bse_d = din("bse", (1, 3), bf16)
    wih_d = din("wiht", (128, 4, 3 * D), bf16)         # [p, kc, f]
    whh_d = din("whht", (128, DC, 3 * D), bf16)
    brz_d = din("brow_rz", (1, 2 * D), bf16)
    bgin_d = din("brow_gin", (1, D), bf16)
    bghn_d = din("brow_ghn", (1, D), bf16)
    ones_d = din("ones128", (1, 128), bf16)
    ones5_d = din("ones512", (1, 512), bf16)
    ones2n_d = din("ones2n", (2, N), bf16)
    if use_mask:
        mt_d = din("maskt", (PER, N, N), bf16)
    out_d = nc.dram_tensor("out", (PER, N, D), f32, kind="ExternalOutput").ap()

    with tile.TileContext(nc) as tc, ExitStack() as ctx:
        if loop_n > 1:
            ctx.enter_context(tc.For_i(0, loop_n, 1))
        wp = ctx.enter_context(tc.tile_pool(name="wp", bufs=1))
        big = ctx.enter_context(tc.tile_pool(name="big", bufs=1))
        rawp = ctx.enter_context(tc.tile_pool(name="rawp", bufs=1))
        atp = ctx.enter_context(tc.tile_pool(name="atp", bufs=33))
        cvp = ctx.enter_context(tc.tile_pool(name="cvp", bufs=2))
        gtp = ctx.enter_context(tc.tile_pool(name="gtp", bufs=1))
        dramp = ctx.enter_context(tc.tile_pool(name="dramp", bufs=1, space="DRAM"))
        ps = ctx.enter_context(tc.tile_pool(name="ps", bufs=4, space="PSUM"))
        pa = ctx.enter_context(tc.tile_pool(name="pa", bufs=2, space="PSUM"))

        # ---- load weights (persistent) ----
        w1 = wp.tile([128, DC, DC, 128], bf16, tag="w1")
        nc.gpsimd.dma_start(w1[:], w1_d[:])
        sh1 = wp.tile([128, DC], f32, tag="sh1")
        nc.gpsimd.dma_start(sh1[:], sh1_d[:])
        sh2 = wp.tile([128, DC], f32, tag="sh2")
        nc.gpsimd.dma_start(sh2[:], sh2_d[:])
        bmsg = wp.tile([1, D], bf16, tag="bmsg")
        nc.gpsimd.dma_start(bmsg[:], bmsg_d[:])
        wsehl = wp.tile([128, DC, 6], bf16, tag="wsehl")
        nc.gpsimd.dma_start(wsehl[:], wsehl_d[:])
        bse = wp.tile([1, 3], bf16, tag="bse")
        nc.gpsimd.dma_start(bse[:], bse_d[:])
        ones = wp.tile([1, 128], bf16, tag="ones")
        nc.gpsimd.dma_start(ones[:], ones_d[:])
        ones5 = wp.tile([1, 512], bf16, tag="ones5")
        nc.gpsimd.dma_start(ones5[:], ones5_d[:])
        # n-major sources for the 13-row RBF factors; rows live in cols 0:13,
        # cols 13:128 zeroed once here (transposed junk would land in unused
        # partitions anyway, but keep the sim's finite-checks happy).
        saSRC = wp.tile([128, NB, 128], bf16, tag="saSRC")
        nc.vector.memset(saSRC[:], 0.0)
        nc.vector.memset(saSRC[:, :, 9:11], 1.0)
        sbSRC = wp.tile([128, NB, 128], bf16, tag="sbSRC")
        nc.vector.memset(sbSRC[:], 0.0)
        nc.vector.memset(sbSRC[:, :, 11:13], 1.0)

        def emit_load(el):
            # ---- load x n-major (contiguous), split bf16 hi/lo, and build
            # channel-major copies via the XBAR transpose DMA (2-byte dtype).
            xnf = rawp.tile([128, NB, D], f32, tag="xnf", bufs=2)
            xnh = rawp.tile([128, NB, D], bf16, tag="xnh", bufs=1)
            xnl = rawp.tile([128, NB, D], bf16, tag="xnl", bufs=1)
            # layout: xth[p_c, nb, dc, p_n] == xT[dc*128+p_c, nb*128+p_n]
            xth = big.tile([128, NB, DC, 128], bf16, tag="xth", bufs=2)
            xtl = big.tile([128, NB, DC, 128], bf16, tag="xtl", bufs=2)
            qn = NB // 4
            quarters = [slice(h * qn, (h + 1) * qn) for h in range(4)]
            for h_, hs in enumerate(quarters):
                nc.sync.dma_start(
                    xnf[:, hs, :],
                    x_d[el, h_ * (N // 4):(h_ + 1) * (N // 4), :].rearrange(
                        "(nb p) d -> p nb d", p=128))
            for hs in quarters:
                nc.vector.tensor_copy(xnh[:, hs, :], xnf[:, hs, :])
                nc.vector.scalar_tensor_tensor(xnl[:, hs, :], xnf[:, hs, :],
                                               1.0, xnh[:, hs, :],
                                               OP.mult, OP.subtract)
            # transposes on the sync HWDGE queue — on the scalar (Act) queue
            # they'd serialize behind the previous element's exp/tanh stream.
            for hs in quarters:
                nc.sync.dma_start_transpose(
                    xth[:, hs].rearrange("p nb dc pn -> p (nb dc) pn"),
                    xnh[:, hs, :])
            for hs in quarters:
                nc.sync.dma_start_transpose(
                    xtl[:, hs].rearrange("p nb dc pn -> p (nb dc) pn"),
                    xnl[:, hs, :])
            return xnf, xth, xtl

        els = [e for _ in range(body_reps) for e in range(PER)]
        pending = {0: emit_load(els[0])}
        for idx, el in enumerate(els):
            xnf, xth, xtl = pending.pop(idx)

            if el == 0:
                w2 = wp.tile([128, DC, KT, DC, 128], fp8, tag="w2")
                nc.sync.dma_start(w2[:], w2_d[:])
                wmsg = wp.tile([128, DC, D], bf16, tag="wmsg")
                nc.sync.dma_start(wmsg[:], wmsg_d[:])
                wih = wp.tile([128, 4, 3 * D], bf16, tag="wih")
                nc.sync.dma_start(wih[:], wih_d[:])
                whh = wp.tile([128, DC, 3 * D], bf16, tag="whh")
                nc.sync.dma_start(whh[:], whh_d[:])
                brz = wp.tile([1, 2 * D], bf16, tag="brz")
                nc.sync.dma_start(brz[:], brz_d[:])
                bgin = wp.tile([1, D], bf16, tag="bgin")
                nc.sync.dma_start(bgin[:], bgin_d[:])
                bghn = wp.tile([1, D], bf16, tag="bghn")
                nc.sync.dma_start(bghn[:], bghn_d[:])
            # ---- s = x @ w_se.T + b_se computed n-major (f32 via hi/lo),
            # 13-row RBF factors assembled wide (128 lanes, free-dim slices
            # only — compute engines can't address odd partition bases), then
            # XBAR-transposed to the [13 rows, n] layout the G matmul needs.
            # Row pairs (SA | SB): 0:3 (2s_hi | s_hi), 3:6 (2s_lo | s_hi),
            # 6:9 (2s_hi | s_lo), 9,10 (1 | -sq_hi, -sq_lo),
            # 11,12 (-sq_hi, -sq_lo | 1).
            saT = big.tile([128, NB, 128], bf16, tag="saT", bufs=1)
            sbT = big.tile([128, NB, 128], bf16, tag="sbT", bufs=1)
            if "se" in skip:
                nc.vector.memset(saT[:], 0.001)
                nc.vector.memset(sbT[:], 0.001)
            else:
                pss = ps.tile([128, 512], f32, tag="ps")
                for nb in range(NB):
                    c6 = slice(6 * nb, 6 * nb + 6)
                    c3 = slice(6 * nb, 6 * nb + 3)
                    nc.tensor.matmul(pss[:, c6], xth[:, nb, 0, :],
                                     wsehl[:, 0, :], start=True, stop=False,
                                     skip_group_check=True)
                    nc.tensor.matmul(pss[:, c6], xth[:, nb, 1, :],
                                     wsehl[:, 1, :], start=False, stop=False,
                                     skip_group_check=True)
                    nc.tensor.matmul(pss[:, c3], xtl[:, nb, 0, :],
                                     wsehl[:, 0, 0:3], start=False, stop=False,
                                     skip_group_check=True)
                    nc.tensor.matmul(pss[:, c3], xtl[:, nb, 1, :],
                                     wsehl[:, 1, 0:3], start=False,
                                     stop=not use_bias, skip_group_check=True)
                    if use_bias:
                        nc.tensor.matmul(pss[:, c3], ones[:], bse[:],
                                         start=False, stop=True,
                                         skip_group_check=True)
                pssv = pss[:, 0:6 * NB].rearrange("p (nb c) -> p nb c", c=6)
                snm = cvp.tile([128, NB, 3], f32, tag="snm", bufs=1)
                nc.vector.tensor_copy(snm[:], pssv[:, :, 3:6])
                nc.vector.tensor_tensor(snm[:], snm[:], pssv[:, :, 0:3], OP.add)
                shi = cvp.tile([128, NB, 3], bf16, tag="shi", bufs=1)
                nc.vector.tensor_copy(shi[:], snm[:])
                slo = cvp.tile([128, NB, 3], bf16, tag="slo", bufs=1)
                nc.vector.scalar_tensor_tensor(slo[:], snm[:], 1.0, shi[:],
                                               OP.mult, OP.subtract)
                ssq = cvp.tile([128, NB, 3], f32, tag="ssq", bufs=1)
                nc.scalar.activation(ssq[:], snm[:], AF.Square)
                sqn = cvp.tile([128, NB, 1], f32, tag="sqn", bufs=1)
                nc.vector.tensor_tensor(sqn[:], ssq[:, :, 0:1], ssq[:, :, 1:2],
                                        OP.add)
                nc.vector.tensor_tensor(sqn[:], sqn[:], ssq[:, :, 2:3], OP.add)
                nc.vector.tensor_scalar(saSRC[:, :, 0:3], shi[:], 2.0, None,
                                        OP.mult)
                nc.vector.tensor_scalar(saSRC[:, :, 3:6], slo[:], 2.0, None,
                                        OP.mult)
                nc.vector.tensor_copy(saSRC[:, :, 6:9], saSRC[:, :, 0:3])
                nc.vector.tensor_scalar(saSRC[:, :, 11:12], sqn[:], -1.0, None,
                                        OP.mult)
                nc.vector.scalar_tensor_tensor(saSRC[:, :, 12:13], sqn[:], -1.0,
                                               saSRC[:, :, 11:12],
                                               OP.mult, OP.subtract)
                nc.vector.tensor_copy(sbSRC[:, :, 0:3], shi[:])
                nc.vector.tensor_copy(sbSRC[:, :, 3:6], shi[:])
                nc.vector.tensor_copy(sbSRC[:, :, 6:9], slo[:])
                nc.vector.tensor_copy(sbSRC[:, :, 9:11], saSRC[:, :, 11:13])
                nc.sync.dma_start_transpose(
                    saT[:], saSRC[:].rearrange("p nb c -> p (nb c)"))
                nc.sync.dma_start_transpose(
                    sbT[:], sbSRC[:].rearrange("p nb c -> p (nb c)"))

            # ---- conv1 (1x1) + bn1 + relu -> h1 (padded by 8 each side) ----
            h1 = big.tile([128, DC, N + 16], fp8, tag="h1", bufs=1)
            nc.vector.memset(h1[:, :, 0:8], 0.0)
            nc.vector.memset(h1[:, :, N + 8:N + 16], 0.0)
            if "conv1" in skip:
                nc.vector.memset(h1[:, :, 8:N + 8], 0.1)
            for mc in range(0 if "conv1" in skip else DC):
                for c in range(LC):
                    pc = ps.tile([128, 512], f32, tag="ps")
                    for kc in range(DC):
                        nc.tensor.matmul(pc[:], w1[:, kc, mc, :],
                                         xth[:, 4 * c:4 * (c + 1), kc, :],
                                         start=(kc == 0), stop=(kc == DC - 1),
                                         skip_group_check=True)
                    nc.scalar.activation(h1[:, mc, 8 + c * 512:8 + (c + 1) * 512],
                                         pc[:], AF.Relu, bias=sh1[:, mc:mc + 1])

            # prefetch next element's load/split/transposes here: DVE and the
            # sync queue are near-idle during conv2, and it keeps the PE
            # stream dense across the element boundary (HAM stays warm).
            if idx + 1 < len(els):
                pending[idx + 1] = emit_load(els[idx + 1])

            # ---- conv2 (17 taps) + bn2 + residual + relu -> x_convT ----
            xcv = big.tile([128, DC, N], bf16, tag="xcv")
            if "conv2" in skip:
                nc.vector.memset(xcv[:], 0.0)
            for mc in range(0 if "conv2" in skip else DC):
                for c in range(LC):
                    pc2 = ps.tile([128, 512], f32, tag="ps")
                    # fp8 DoubleRow: one matmul contracts both kc blocks
                    # (K=256) — 17 MMs per chunk instead of 34 at ~1.4x rate.
                    for t in range(KT):
                        nc.tensor.matmul(
                            pc2[:], w2[:, :, t, mc, :],
                            h1[:, :, c * 512 + t:c * 512 + t + 512],
                            start=(t == 0), stop=(t == KT - 1),
                            perf_mode=mybir.MatmulPerfMode.DoubleRow,
                            skip_group_check=True)
                    tv = cvp.tile([128, 4, 128], f32, tag="cv", bufs=2)
                    nc.vector.tensor_tensor(tv[:], pc2[:].rearrange(
                        "p (a b) -> p a b", b=128),
                        xth[:, 4 * c:4 * (c + 1), mc, :], OP.add)
                    nc.vector.tensor_tensor(tv[:], tv[:],
                                            xtl[:, 4 * c:4 * (c + 1), mc, :],
                                            OP.add)
                    nc.scalar.activation(
                        xcv[:, mc, c * 512:(c + 1) * 512],
                        tv[:].rearrange("p a b -> p (a b)"),
                        AF.Relu, bias=sh2[:, mc:mc + 1])

            # ---- msg = relu(x @ w_msg.T + b_msg), n-major ----
            msg = big.tile([128, NB, D], bf16, tag="msg")
            if "msg" in skip:
                nc.vector.memset(msg[:], 0.01)
            for nb in range(0 if "msg" in skip else NB):
                pm = ps.tile([128, 512], f32, tag="ps")
                for kc in range(DC):
                    nc.tensor.matmul(pm[:, 0:D], xth[:, nb, kc, :],
                                     wmsg[:, kc, :], start=(kc == 0),
                                     stop=(not use_bias and kc == DC - 1),
                                     skip_group_check=True)
                if use_bias:
                    nc.tensor.matmul(pm[:, 0:D], ones[:], bmsg[:], start=False,
                                     stop=True, skip_group_check=True)
                nc.scalar.activation(msg[:, nb, :], pm[:, 0:D], AF.Relu)

            # ---- A-branch: x_nmpT[d, i] = sum_j exp(-dist[j,i]) * msg[j, d] ----
            xnm = big.tile([128, DC, N], bf16, tag="xnm")
            if "noG" in skip and el == 0:
                atc = wp.tile([128, 512], bf16, tag="atc")
                nc.vector.memset(atc[:], 0.001)
            if "A" in skip:
                nc.vector.memset(xnm[:], 0.0)

            # A-branch restructured into long clean matmul runs (conv2-style):
            # per i-chunk, batch all 16 G matmuls + exps, then run the 16
            # m0-accumulations back-to-back into one bank, then the 16 m1 —
            # software-pipelined one chunk deep (acc of chunk ic-1 overlaps
            # exp of chunk ic).  Interleaved short groups measured ~560 ns/MM
            # on HW vs ~240 ns/MM for long runs.
            at_store = {}

            def emit_acc(ic):
                accs = [pa.tile([128, 512], f32, tag="acc0", name="a0"),
                        pa.tile([128, 512], f32, tag="acc1", name="a1")]
                for mc in range(DC):
                    for jb in range(NB):
                        nc.tensor.matmul(accs[mc][:],
                                         msg[:, jb, mc * 128:(mc + 1) * 128],
                                         at_store[(ic, jb)][:],
                                         start=(jb == 0), stop=(jb == NB - 1),
                                         skip_group_check=True)
                for mc in range(DC):
                    nc.vector.tensor_copy(xnm[:, mc, ic * 512:(ic + 1) * 512],
                                          accs[mc][:])

            for ic in range(0 if "A" in skip else LC):
                for jb in range(NB):
                    if "noG" in skip:
                        at_store[(ic, jb)] = atc
                        continue
                    pgm = ps.tile([128, 512], f32, tag="ps")
                    nc.tensor.matmul(pgm[:], saT[0:13, jb, :],
                                     sbT[0:13, 4 * ic:4 * (ic + 1), :],
                                     start=True, stop=True,
                                     skip_group_check=True)
                    at = atp.tile([128, 512], bf16, tag="at")
                    if "exp2dve" in skip:
                        nc.vector.tensor_copy(at[:], pgm[:])
                    else:
                        nc.scalar.activation(at[:], pgm[:], AF.Exp)
                    if use_mask:
                        mtt = cvp.tile([128, 512], bf16, tag="mtt")
                        nc.sync.dma_start(mtt[:],
                                          mt_d[el, jb * 128:(jb + 1) * 128,
                                               ic * 512:(ic + 1) * 512])
                        nc.vector.tensor_tensor(at[:], at[:], mtt[:], OP.mult)
                    at_store[(ic, jb)] = at
                if ic >= 1:
                    emit_acc(ic - 1)
            if "A" not in skip:
                emit_acc(LC - 1)

            # ---- GRU gates (n-major) ----
            if "gru" in skip:
                for nb in range(NB):
                    sl = slice(nb * 128, (nb + 1) * 128)
                    nc.sync.dma_start(out_d[el, sl, :], xnf[:, nb, :])
            for nb in range(0 if "gru" in skip else NB):
                sl = slice(nb * 128, (nb + 1) * 128)
                prz = ps.tile([128, 512], f32, tag="ps")
                ih_srcs = [xcv[:, 0, sl], xcv[:, 1, sl], xnm[:, 0, sl], xnm[:, 1, sl]]
                hh_srcs = [xth[:, nb, 0, :], xth[:, nb, 1, :]]
                for ci in range(4):
                    nc.tensor.matmul(prz[:], ih_srcs[ci], wih[:, ci, 0:512],
                                     start=(ci == 0), stop=False,
                                     skip_group_check=True)
                for kc in range(DC):
                    nc.tensor.matmul(prz[:], hh_srcs[kc], whh[:, kc, 0:512],
                                     start=False,
                                     stop=(not use_bias and kc == DC - 1),
                                     skip_group_check=True)
                if use_bias:
                    nc.tensor.matmul(prz[:], ones[:], brz[:], start=False,
                                     stop=True, skip_group_check=True)

                # gi_n in cols 0:D, gh_n in cols D:2D of ONE psum tile —
                # 2 psum allocs per block instead of 3, so two blocks pipeline
                pgg = ps.tile([128, 512], f32, tag="ps")
                for ci in range(4):
                    nc.tensor.matmul(pgg[:, 0:D], ih_srcs[ci], wih[:, ci, 512:768],
                                     start=(ci == 0),
                                     stop=(not use_bias and ci == 3),
                                     skip_group_check=True)
                if use_bias:
                    nc.tensor.matmul(pgg[:, 0:D], ones[:], bgin[:], start=False,
                                     stop=True, skip_group_check=True)
                for kc in range(DC):
                    nc.tensor.matmul(pgg[:, D:2 * D], hh_srcs[kc],
                                     whh[:, kc, 512:768], start=(kc == 0),
                                     stop=(not use_bias and kc == DC - 1),
                                     skip_group_check=True)
                if use_bias:
                    nc.tensor.matmul(pgg[:, D:2 * D], ones[:], bghn[:],
                                     start=False, stop=True,
                                     skip_group_check=True)

                # w_hh n-gate columns are pre-scaled by 0.5 on the host, so
                # pgg[:, D:2D] = 0.5*gh_n =: g.  With tr = tanh(0.5*(r-gate)):
                #   r*gh_n = ((tr+1)/2)*gh_n = (tr+1)*g
                #   out    = nn + z*(x-nn) = nn + 0.5*(tz+1)*(x-nn)
                tr = gtp.tile([128, D], f32, tag="tr", bufs=2)
                nc.scalar.activation(tr[:], prz[:, 0:D], AF.Tanh, scale=0.5)
                tz = gtp.tile([128, D], f32, tag="tz", bufs=2)
                nc.scalar.activation(tz[:], prz[:, D:2 * D], AF.Tanh, scale=0.5)
                q = gtp.tile([128, D], f32, tag="q", bufs=1)
                nc.vector.scalar_tensor_tensor(q[:], tr[:], 1.0, pgg[:, D:2 * D],
                                               OP.add, OP.mult)
                nc.vector.tensor_tensor(q[:], q[:], pgg[:, 0:D], OP.add)
                nn = gtp.tile([128, D], f32, tag="nn", bufs=2)
                nc.scalar.activation(nn[:], q[:], AF.Tanh)
                dd = gtp.tile([128, D], f32, tag="dd", bufs=1)
                nc.vector.tensor_tensor(dd[:], xnf[:, nb, :], nn[:], OP.subtract)
                nc.vector.scalar_tensor_tensor(dd[:], tz[:], 1.0, dd[:],
                                               OP.add, OP.mult)
                ho = gtp.tile([128, D], f32, tag="ho", bufs=2)
                nc.vector.scalar_tensor_tensor(ho[:], dd[:], 0.5, nn[:],
                                               OP.mult, OP.add)
                nc.sync.dma_start(out_d[el, sl, :], ho[:])

    nc.compile()
    return nc


def _host_prep(inputs):
    g = {k: np.asarray(v, np.float32) for k, v in inputs.items()}
    sc1 = g["bn1_g"] / np.sqrt(g["bn1_v"] + EPS)
    sh1 = g["bn1_b"] - g["bn1_m"] * sc1
    sc2 = g["bn2_g"] / np.sqrt(g["bn2_v"] + EPS)
    sh2 = g["bn2_b"] - g["bn2_m"] * sc2

    w1p = g["conv1_w"][:, :, 0] * sc1[:, None]          # (O, I)
    w2p = g["conv2_w"] * sc2[:, None, None]             # (O, I, 17)

    def lhsT_pack(w):   # (O, I) -> (128, kc=I/128, mc=O/128, 128): [p,kc,mc,m]
        o, i = w.shape
        return np.ascontiguousarray(np.transpose(
            w.T.reshape(i // 128, 128, o // 128, 128), (1, 0, 2, 3)))

    w1t = lhsT_pack(w1p).astype(BF)
    w2t = np.stack([lhsT_pack(w2p[:, :, t]) for t in range(KT)], axis=2)
    w2t = np.ascontiguousarray(np.transpose(w2t, (0, 1, 2, 3, 4)))  # [p,kc,t,mc,m]
    w2t = w2t.astype(E4)

    def rhs_pack(wt):   # (Kdim, F) -> (128, kc, F)
        k, f = wt.shape
        return np.ascontiguousarray(
            np.transpose(wt.reshape(k // 128, 128, f), (1, 0, 2)))

    wmsgt = rhs_pack(g["w_msg"].T).astype(BF)
    wiht = rhs_pack(g["w_ih"].T).astype(BF)
    whh_t = g["w_hh"].T.copy()                           # (256, 768)
    whh_t[:, 2 * D:] *= 0.5                              # n-gate pre-scaled
    whht = rhs_pack(whh_t).astype(BF)

    wse_t = g["w_se"].T                                  # (256, 3)
    wse_hi = wse_t.astype(BF)
    wse_lo = (wse_t - wse_hi.astype(np.float32)).astype(BF)
    wsehl = rhs_pack(np.concatenate(
        [wse_hi.astype(np.float32), wse_lo.astype(np.float32)],
        axis=1)).astype(BF)

    bih, bhh = g["b_ih"], g["b_hh"]
    feed = {
        "w1t": w1t, "w2t": w2t,
        "sh1": np.ascontiguousarray(sh1.reshape(DC, 128).T.astype(np.float32)),
        "sh2": np.ascontiguousarray(sh2.reshape(DC, 128).T.astype(np.float32)),
        "wmsgt": wmsgt, "bmsg": g["b_msg"].reshape(1, D).astype(BF),
        "wsehl": wsehl,
        "bse": g["b_se"].reshape(1, 3).astype(BF),
        "wiht": wiht, "whht": whht,
        "brow_rz": (bih[:2 * D] + bhh[:2 * D]).reshape(1, 2 * D).astype(BF),
        "brow_gin": bih[2 * D:].reshape(1, D).astype(BF),
        "brow_ghn": (0.5 * bhh[2 * D:]).reshape(1, D).astype(BF),
        "ones128": np.ones((1, 128), BF),
        "ones512": np.ones((1, 512), BF),
        "ones2n": np.ones((2, N), BF),
    }
    return g, feed


def make_in_maps(inputs):
    g, feed = _host_prep(inputs)
    x = g["x"]
    mask = g["mask"]
    use_mask = not bool(np.all(mask == 1.0))
    use_bias = not (np.all(g["b_se"] == 0) and np.all(g["b_msg"] == 0)
                    and np.all(g["b_ih"] == 0) and np.all(g["b_hh"] == 0))
    in_maps = []
    for i in range(NCORE):
        m = dict(feed)
        m["x"] = np.ascontiguousarray(x[i * PER:(i + 1) * PER])
        if use_mask:
            m["maskt"] = np.ascontiguousarray(
                mask[i * PER:(i + 1) * PER].transpose(0, 2, 1)).astype(BF)
        in_maps.append(m)
    return in_maps, use_mask, use_bias


def get_nc(use_mask: bool, use_bias: bool = True):
    key = (use_mask, use_bias)
    if key not in _built:
        _built[key] = _build(use_mask, use_bias)
    return _built[key]


def kernel(**inputs) -> np.ndarray:
    in_maps, use_mask, use_bias = make_in_maps(inputs)
    nc = get_nc(use_mask, use_bias)
    from concourse import bass_utils
    last_err = None
    for attempt in range(3):
        try:
            res = bass_utils.run_bass_kernel_spmd(nc, in_maps,
                                                  core_ids=list(range(NCORE)))
            out = np.concatenate([res.results[i]["out"] for i in range(NCORE)],
                                 axis=0)
            return np.ascontiguousarray(out.astype(np.float32))
        except Exception as e:  # wedged device: reset backend and retry
            last_err = e
            try:
                import jax
                jax.clear_caches()
                jax.extend.backend.clear_backends()
            except Exception:
                pass
            import time as _t
            _t.sleep(5)
    raise last_err



# revision 28
# speedup vs baseline: 2.0663x; 1.1061x over previous
"""Trainium2 Bass kernel for nn_ConvolutionalNMPBlock.

Self-contained: takes full (unsharded) inputs, shards batch across 8
NeuronCores (2 elements each), runs a fused Bass/Tile kernel, gathers.
"""
import numpy as np
import ml_dtypes

BS, N, D = 16, 2048, 256
NCORE = 8
PER = BS // NCORE          # batch elements per core
EPS = 1e-5
NB = N // 128              # 16 row blocks
LC = N // 512              # 4 column chunks of 512
DC = D // 128              # 2 channel blocks
KT = 17                    # conv2 taps
BF = ml_dtypes.bfloat16
E4 = ml_dtypes.float8_e4m3

_built = {}                # use_mask -> compiled nc


def _build(use_mask: bool, use_bias: bool = True, loop_n: int = 1,
           skip: frozenset = frozenset(), body_reps: int = 1):
    from concourse import bacc, tile
    import concourse.mybir as mybir
    from contextlib import ExitStack

    f32 = mybir.dt.float32
    bf16 = mybir.dt.bfloat16
    AF = mybir.ActivationFunctionType
    OP = mybir.AluOpType

    nc = bacc.Bacc("TRN2", target_bir_lowering=False, debug=False,
                   num_devices=NCORE)

    def din(name, shape, dt=f32):
        return nc.dram_tensor(name, shape, dt, kind="ExternalInput").ap()

    fp8 = mybir.dt.float8e4
    x_d = din("x", (PER, N, D))
    w1_d = din("w1t", (128, DC, DC, 128), bf16)        # [p=kin, kc, mc, m]
    w2_d = din("w2t", (128, DC, KT, DC, 128), fp8)     # [p, kc, tap, mc, m]
    sh1_d = din("sh1", (128, DC))
    sh2_d = din("sh2", (128, DC))
    wmsg_d = din("wmsgt", (128, DC, D), bf16)          # [p, kc, f]
    bmsg_d = din("bmsg", (1, D), bf16)
    wsehl_d = din("wsehl", (128, DC, 6), bf16)         # [p, kc, hi(3)|lo(3)]
    bse_d = din("bse", (1, 3), bf16)
    wih_d = din("wiht", (128, 4, 3 * D), bf16)         # [p, kc, f]
    whh_d = din("whht", (128, DC, 3 * D), bf16)
    brz_d = din("brow_rz", (1, 2 * D), bf16)
    bgin_d = din("brow_gin", (1, D), bf16)
    bghn_d = din("brow_ghn", (1, D), bf16)
    ones_d = din("ones128", (1, 128), bf16)
    ones5_d = din("ones512", (1, 512), bf16)
    ones2n_d = din("ones2n", (2, N), bf16)
    if use_mask:
        mt_d = din("maskt", (PER, N, N), bf16)
    out_d = nc.dram_tensor("out", (PER, N, D), f32, kind="ExternalOutput").ap()

    with tile.TileContext(nc) as tc, ExitStack() as ctx:
        if loop_n > 1:
            ctx.enter_context(tc.For_i(0, loop_n, 1))
        wp = ctx.enter_context(tc.tile_pool(name="wp", bufs=1))
        big = ctx.enter_context(tc.tile_pool(name="big", bufs=1))
        rawp = ctx.enter_context(tc.tile_pool(name="rawp", bufs=1))
        atp = ctx.enter_context(tc.tile_pool(name="atp", bufs=33))
        cvp = ctx.enter_context(tc.tile_pool(name="cvp", bufs=2))
        gtp = ctx.enter_context(tc.tile_pool(name="gtp", bufs=1))
        dramp = ctx.enter_context(tc.tile_pool(name="dramp", bufs=1, space="DRAM"))
        ps = ctx.enter_context(tc.tile_pool(name="ps", bufs=6, space="PSUM"))
        pa = ctx.enter_context(tc.tile_pool(name="pa", bufs=1, space="PSUM"))

        # ---- load weights (persistent) ----
        w1 = wp.tile([128, DC, DC, 128], bf16, tag="w1")
        nc.gpsimd.dma_start(w1[:], w1_d[:])
        sh1 = wp.tile([128, DC], f32, tag="sh1")
        nc.gpsimd.dma_start(sh1[:], sh1_d[:])
        sh2 = wp.tile([128, DC], f32, tag="sh2")
        nc.gpsimd.dma_start(sh2[:], sh2_d[:])
        bmsg = wp.tile([1, D], bf16, tag="bmsg")
        nc.gpsimd.dma_start(bmsg[:], bmsg_d[:])
        wsehl = wp.tile([128, DC, 6], bf16, tag="wsehl")
        nc.gpsimd.dma_start(wsehl[:], wsehl_d[:])
        bse = wp.tile([1, 3], bf16, tag="bse")
        nc.gpsimd.dma_start(bse[:], bse_d[:])
        ones = wp.tile([1, 128], bf16, tag="ones")
        nc.gpsimd.dma_start(ones[:], ones_d[:])
        ones5 = wp.tile([1, 512], bf16, tag="ones5")
        nc.gpsimd.dma_start(ones5[:], ones5_d[:])
        # n-major sources for the 13-row RBF factors; rows live in cols 0:13,
        # cols 13:128 zeroed once here (transposed junk would land in unused
        # partitions anyway, but keep the sim's finite-checks happy).
        saSRC = wp.tile([128, NB, 128], bf16, tag="saSRC")
        nc.vector.memset(saSRC[:], 0.0)
        nc.vector.memset(saSRC[:, :, 9:11], 1.0)
        sbSRC = wp.tile([128, NB, 128], bf16, tag="sbSRC")
        nc.vector.memset(sbSRC[:], 0.0)
        nc.vector.memset(sbSRC[:, :, 11:13], 1.0)

        def emit_load(el):
            # ---- load x n-major (contiguous), split bf16 hi/lo, and build
            # channel-major copies via the XBAR transpose DMA (2-byte dtype).
            xnf = rawp.tile([128, NB, D], f32, tag="xnf", bufs=2)
            xnh = rawp.tile([128, NB, D], bf16, tag="xnh", bufs=1)
            xnl = rawp.tile([128, NB, D], bf16, tag="xnl", bufs=1)
            # layout: xth[p_c, nb, dc, p_n] == xT[dc*128+p_c, nb*128+p_n]
            xth = big.tile([128, NB, DC, 128], bf16, tag="xth", bufs=2)
            xtl = big.tile([128, NB, DC, 128], bf16, tag="xtl", bufs=2)
            qn = NB // 4
            quarters = [slice(h * qn, (h + 1) * qn) for h in range(4)]
            for h_, hs in enumerate(quarters):
                nc.sync.dma_start(
                    xnf[:, hs, :],
                    x_d[el, h_ * (N // 4):(h_ + 1) * (N // 4), :].rearrange(
                        "(nb p) d -> p nb d", p=128))
            for hs in quarters:
                nc.vector.tensor_copy(xnh[:, hs, :], xnf[:, hs, :])
                nc.vector.scalar_tensor_tensor(xnl[:, hs, :], xnf[:, hs, :],
                                               1.0, xnh[:, hs, :],
                                               OP.mult, OP.subtract)
            # transposes on the sync HWDGE queue — on the scalar (Act) queue
            # they'd serialize behind the previous element's exp/tanh stream.
            for hs in quarters:
                nc.sync.dma_start_transpose(
                    xth[:, hs].rearrange("p nb dc pn -> p (nb dc) pn"),
                    xnh[:, hs, :])
            for hs in quarters:
                nc.sync.dma_start_transpose(
                    xtl[:, hs].rearrange("p nb dc pn -> p (nb dc) pn"),
                    xnl[:, hs, :])
            return xnf, xth, xtl

        els = [e for _ in range(body_reps) for e in range(PER)]
        pending = {0: emit_load(els[0])}
        for idx, el in enumerate(els):
            xnf, xth, xtl = pending.pop(idx)

            if el == 0:
                w2 = wp.tile([128, DC, KT, DC, 128], fp8, tag="w2")
                nc.sync.dma_start(w2[:], w2_d[:])
                wmsg = wp.tile([128, DC, D], bf16, tag="wmsg")
                nc.sync.dma_start(wmsg[:], wmsg_d[:])
                wih = wp.tile([128, 4, 3 * D], bf16, tag="wih")
                nc.sync.dma_start(wih[:], wih_d[:])
                whh = wp.tile([128, DC, 3 * D], bf16, tag="whh")
                nc.sync.dma_start(whh[:], whh_d[:])
                brz = wp.tile([1, 2 * D], bf16, tag="brz")
                nc.sync.dma_start(brz[:], brz_d[:])
                bgin = wp.tile([1, D], bf16, tag="bgin")
                nc.sync.dma_start(bgin[:], bgin_d[:])
                bghn = wp.tile([1, D], bf16, tag="bghn")
                nc.sync.dma_start(bghn[:], bghn_d[:])
            # ---- s = x @ w_se.T + b_se computed n-major (f32 via hi/lo),
            # 13-row RBF factors assembled wide (128 lanes, free-dim slices
            # only — compute engines can't address odd partition bases), then
            # XBAR-transposed to the [13 rows, n] layout the G matmul needs.
            # Row pairs (SA | SB): 0:3 (2s_hi | s_hi), 3:6 (2s_lo | s_hi),
            # 6:9 (2s_hi | s_lo), 9,10 (1 | -sq_hi, -sq_lo),
            # 11,12 (-sq_hi, -sq_lo | 1).
            saT = big.tile([128, NB, 128], bf16, tag="saT", bufs=1)
            sbT = big.tile([128, NB, 128], bf16, tag="sbT", bufs=1)
            if "se" in skip:
                nc.vector.memset(saT[:], 0.001)
                nc.vector.memset(sbT[:], 0.001)
            else:
                pss = ps.tile([128, 512], f32, tag="ps")
                for nb in range(NB):
                    c6 = slice(6 * nb, 6 * nb + 6)
                    c3 = slice(6 * nb, 6 * nb + 3)
                    nc.tensor.matmul(pss[:, c6], xth[:, nb, 0, :],
                                     wsehl[:, 0, :], start=True, stop=False,
                                     skip_group_check=True)
                    nc.tensor.matmul(pss[:, c6], xth[:, nb, 1, :],
                                     wsehl[:, 1, :], start=False, stop=False,
                                     skip_group_check=True)
                    nc.tensor.matmul(pss[:, c3], xtl[:, nb, 0, :],
                                     wsehl[:, 0, 0:3], start=False, stop=False,
                                     skip_group_check=True)
                    nc.tensor.matmul(pss[:, c3], xtl[:, nb, 1, :],
                                     wsehl[:, 1, 0:3], start=False,
                                     stop=not use_bias, skip_group_check=True)
                    if use_bias:
                        nc.tensor.matmul(pss[:, c3], ones[:], bse[:],
                                         start=False, stop=True,
                                         skip_group_check=True)
                pssv = pss[:, 0:6 * NB].rearrange("p (nb c) -> p nb c", c=6)
                snm = cvp.tile([128, NB, 3], f32, tag="snm", bufs=1)
                nc.vector.tensor_copy(snm[:], pssv[:, :, 3:6])
                nc.vector.tensor_tensor(snm[:], snm[:], pssv[:, :, 0:3], OP.add)
                shi = cvp.tile([128, NB, 3], bf16, tag="shi", bufs=1)
                nc.vector.tensor_copy(shi[:], snm[:])
                slo = cvp.tile([128, NB, 3], bf16, tag="slo", bufs=1)
                nc.vector.scalar_tensor_tensor(slo[:], snm[:], 1.0, shi[:],
                                               OP.mult, OP.subtract)
                ssq = cvp.tile([128, NB, 3], f32, tag="ssq", bufs=1)
                nc.scalar.activation(ssq[:], snm[:], AF.Square)
                sqn = cvp.tile([128, NB, 1], f32, tag="sqn", bufs=1)
                nc.vector.tensor_tensor(sqn[:], ssq[:, :, 0:1], ssq[:, :, 1:2],
                                        OP.add)
                nc.vector.tensor_tensor(sqn[:], sqn[:], ssq[:, :, 2:3], OP.add)
                nc.vector.tensor_scalar(saSRC[:, :, 0:3], shi[:], 2.0, None,
                                        OP.mult)
                nc.vector.tensor_scalar(saSRC[:, :, 3:6], slo[:], 2.0, None,
                                        OP.mult)
                nc.vector.tensor_copy(saSRC[:, :, 6:9], saSRC[:, :, 0:3])
                nc.vector.tensor_scalar(saSRC[:, :, 11:12], sqn[:], -1.0, None,
                                        OP.mult)
                nc.vector.scalar_tensor_tensor(saSRC[:, :, 12:13], sqn[:], -1.0,
                                               saSRC[:, :, 11:12],
                                               OP.mult, OP.subtract)
                nc.vector.tensor_copy(sbSRC[:, :, 0:3], shi[:])
                nc.vector.tensor_copy(sbSRC[:, :, 3:6], shi[:])
                nc.vector.tensor_copy(sbSRC[:, :, 6:9], slo[:])
                nc.vector.tensor_copy(sbSRC[:, :, 9:11], saSRC[:, :, 11:13])
                nc.sync.dma_start_transpose(
                    saT[:], saSRC[:].rearrange("p nb c -> p (nb c)"))
                nc.sync.dma_start_transpose(
                    sbT[:], sbSRC[:].rearrange("p nb c -> p (nb c)"))

            # ---- conv1 (1x1) + bn1 + relu -> h1 (padded by 8 each side) ----
            h1 = big.tile([128, DC, N + 16], fp8, tag="h1", bufs=1)
            nc.vector.memset(h1[:, :, 0:8], 0.0)
            nc.vector.memset(h1[:, :, N + 8:N + 16], 0.0)
            if "conv1" in skip:
                nc.vector.memset(h1[:, :, 8:N + 8], 0.1)
            for mc in range(0 if "conv1" in skip else DC):
                for c in range(LC):
                    pc = ps.tile([128, 512], f32, tag="ps")
                    for kc in range(DC):
                        nc.tensor.matmul(pc[:], w1[:, kc, mc, :],
                                         xth[:, 4 * c:4 * (c + 1), kc, :],
                                         start=(kc == 0), stop=(kc == DC - 1),
                                         skip_group_check=True)
                    nc.scalar.activation(h1[:, mc, 8 + c * 512:8 + (c + 1) * 512],
                                         pc[:], AF.Relu, bias=sh1[:, mc:mc + 1])

            # prefetch next element's load/split/transposes here: DVE and the
            # sync queue are near-idle during conv2, and it keeps the PE
            # stream dense across the element boundary (HAM stays warm).
            if idx + 1 < len(els):
                pending[idx + 1] = emit_load(els[idx + 1])

            # ---- conv2 (17 taps) + bn2 + residual + relu -> x_convT ----
            xcv = big.tile([128, DC, N], bf16, tag="xcv")
            if "conv2" in skip:
                nc.vector.memset(xcv[:], 0.0)
            for mc in range(0 if "conv2" in skip else DC):
                for c in range(LC):
                    pc2 = ps.tile([128, 512], f32, tag="ps")
                    # fp8 DoubleRow: one matmul contracts both kc blocks
                    # (K=256) — 17 MMs per chunk instead of 34 at ~1.4x rate.
                    for t in range(KT):
                        nc.tensor.matmul(
                            pc2[:], w2[:, :, t, mc, :],
                            h1[:, :, c * 512 + t:c * 512 + t + 512],
                            start=(t == 0), stop=(t == KT - 1),
                            perf_mode=mybir.MatmulPerfMode.DoubleRow,
                            skip_group_check=True)
                    tv = cvp.tile([128, 4, 128], f32, tag="cv", bufs=2)
                    nc.vector.tensor_tensor(tv[:], pc2[:].rearrange(
                        "p (a b) -> p a b", b=128),
                        xth[:, 4 * c:4 * (c + 1), mc, :], OP.add)
                    nc.vector.tensor_tensor(tv[:], tv[:],
                                            xtl[:, 4 * c:4 * (c + 1), mc, :],
                                            OP.add)
                    nc.scalar.activation(
                        xcv[:, mc, c * 512:(c + 1) * 512],
                        tv[:].rearrange("p a b -> p (a b)"),
                        AF.Relu, bias=sh2[:, mc:mc + 1])

            # ---- msg = relu(x @ w_msg.T + b_msg), n-major ----
            msg = big.tile([128, NB, D], bf16, tag="msg")
            if "msg" in skip:
                nc.vector.memset(msg[:], 0.01)
            for nb in range(0 if "msg" in skip else NB):
                pm = ps.tile([128, 512], f32, tag="ps")
                for kc in range(DC):
                    nc.tensor.matmul(pm[:, 0:D], xth[:, nb, kc, :],
                                     wmsg[:, kc, :], start=(kc == 0),
                                     stop=(not use_bias and kc == DC - 1),
                                     skip_group_check=True)
                if use_bias:
                    nc.tensor.matmul(pm[:, 0:D], ones[:], bmsg[:], start=False,
                                     stop=True, skip_group_check=True)
                nc.scalar.activation(msg[:, nb, :], pm[:, 0:D], AF.Relu)

            # ---- A-branch: x_nmpT[d, i] = sum_j exp(-dist[j,i]) * msg[j, d] ----
            xnm = big.tile([128, DC, N], bf16, tag="xnm")
            if "noG" in skip and el == 0:
                atc = wp.tile([128, 512], bf16, tag="atc")
                nc.vector.memset(atc[:], 0.001)
            if "A" in skip:
                nc.vector.memset(xnm[:], 0.0)

            # A-branch restructured into long clean matmul runs (conv2-style):
            # per i-chunk, batch all 16 G matmuls + exps, then run the 16
            # m0-accumulations back-to-back into one bank, then the 16 m1 —
            # software-pipelined one chunk deep (acc of chunk ic-1 overlaps
            # exp of chunk ic).  Interleaved short groups measured ~560 ns/MM
            # on HW vs ~240 ns/MM for long runs.
            at_store = {}

            def emit_acc(ic):
                accs = [pa.tile([128, 512], f32, tag="acc0", name="a0"),
                        pa.tile([128, 512], f32, tag="acc1", name="a1")]
                for mc in range(DC):
                    for jb in range(NB):
                        nc.tensor.matmul(accs[mc][:],
                                         msg[:, jb, mc * 128:(mc + 1) * 128],
                                         at_store[(ic, jb)][:],
                                         start=(jb == 0), stop=(jb == NB - 1),
                                         skip_group_check=True)
                for mc in range(DC):
                    nc.vector.tensor_copy(xnm[:, mc, ic * 512:(ic + 1) * 512],
                                          accs[mc][:])

            for ic in range(0 if "A" in skip else LC):
                for jb in range(NB):
                    if "noG" in skip:
                        at_store[(ic, jb)] = atc
                        continue
                    pgm = ps.tile([128, 512], f32, tag="ps")
                    nc.tensor.matmul(pgm[:], saT[0:13, jb, :],
                                     sbT[0:13, 4 * ic:4 * (ic + 1), :],
                                     start=True, stop=True,
                                     skip_group_check=True)
                    at = atp.tile([128, 512], bf16, tag="at")
                    if "exp2dve" in skip:
                        nc.vector.tensor_copy(at[:], pgm[:])
                    else:
                        nc.scalar.activation(at[:], pgm[:], AF.Exp)
                    if use_mask:
                        mtt = cvp.tile([128, 512], bf16, tag="mtt")
                        nc.sync.dma_start(mtt[:],
                                          mt_d[el, jb * 128:(jb + 1) * 128,
                                               ic * 512:(ic + 1) * 512])
                        nc.vector.tensor_tensor(at[:], at[:], mtt[:], OP.mult)
                    at_store[(ic, jb)] = at
                if ic >= 1:
                    emit_acc(ic - 1)
            if "A" not in skip:
                emit_acc(LC - 1)

            # ---- GRU gates (n-major) ----
            if "gru" in skip:
                for nb in range(NB):
                    sl = slice(nb * 128, (nb + 1) * 128)
                    nc.sync.dma_start(out_d[el, sl, :], xnf[:, nb, :])
            for nb in range(0 if "gru" in skip else NB):
                sl = slice(nb * 128, (nb + 1) * 128)
                prz = ps.tile([128, 512], f32, tag="ps")
                ih_srcs = [xcv[:, 0, sl], xcv[:, 1, sl], xnm[:, 0, sl], xnm[:, 1, sl]]
                hh_srcs = [xth[:, nb, 0, :], xth[:, nb, 1, :]]
                for ci in range(4):
                    nc.tensor.matmul(prz[:], ih_srcs[ci], wih[:, ci, 0:512],
                                     start=(ci == 0), stop=False,
                                     skip_group_check=True)
                for kc in range(DC):
                    nc.tensor.matmul(prz[:], hh_srcs[kc], whh[:, kc, 0:512],
                                     start=False,
                                     stop=(not use_bias and kc == DC - 1),
                                     skip_group_check=True)
                if use_bias:
                    nc.tensor.matmul(prz[:], ones[:], brz[:], start=False,
                                     stop=True, skip_group_check=True)

                # gi_n in cols 0:D, gh_n in cols D:2D of ONE psum tile —
                # 2 psum allocs per block instead of 3, so two blocks pipeline
                pgg = ps.tile([128, 512], f32, tag="ps")
                for ci in range(4):
                    nc.tensor.matmul(pgg[:, 0:D], ih_srcs[ci], wih[:, ci, 512:768],
                                     start=(ci == 0),
                                     stop=(not use_bias and ci == 3),
                                     skip_group_check=True)
                if use_bias:
                    nc.tensor.matmul(pgg[:, 0:D], ones[:], bgin[:], start=False,
                                     stop=True, skip_group_check=True)
                for kc in range(DC):
                    nc.tensor.matmul(pgg[:, D:2 * D], hh_srcs[kc],
                                     whh[:, kc, 512:768], start=(kc == 0),
                                     stop=(not use_bias and kc == DC - 1),
                                     skip_group_check=True)
                if use_bias:
                    nc.tensor.matmul(pgg[:, D:2 * D], ones[:], bghn[:],
                                     start=False, stop=True,
                                     skip_group_check=True)

                # w_hh n-gate columns are pre-scaled by 0.5 on the host, so
                # pgg[:, D:2D] = 0.5*gh_n =: g.  With tr = tanh(0.5*(r-gate)):
                #   r*gh_n = ((tr+1)/2)*gh_n = (tr+1)*g
                #   out    = nn + z*(x-nn) = nn + 0.5*(tz+1)*(x-nn)
                tr = gtp.tile([128, D], f32, tag="tr", bufs=2)
                nc.scalar.activation(tr[:], prz[:, 0:D], AF.Tanh, scale=0.5)
                tz = gtp.tile([128, D], f32, tag="tz", bufs=2)
                nc.scalar.activation(tz[:], prz[:, D:2 * D], AF.Tanh, scale=0.5)
                q = gtp.tile([128, D], f32, tag="q", bufs=1)
                nc.vector.scalar_tensor_tensor(q[:], tr[:], 1.0, pgg[:, D:2 * D],
                                               OP.add, OP.mult)
                nc.vector.tensor_tensor(q[:], q[:], pgg[:, 0:D], OP.add)
                nn = gtp.tile([128, D], f32, tag="nn", bufs=2)
                nc.scalar.activation(nn[:], q[:], AF.Tanh)
                dd = gtp.tile([128, D], f32, tag="dd", bufs=1)
                nc.vector.tensor_tensor(dd[:], xnf[:, nb, :], nn[:], OP.subtract)
                nc.vector.scalar_tensor_tensor(dd[:], tz[:], 1.0, dd[:],
                                               OP.add, OP.mult)
                ho = gtp.tile([128, D], f32, tag="ho", bufs=2)
                nc.vector.scalar_tensor_tensor(ho[:], dd[:], 0.5, nn[:],
                                               OP.mult, OP.add)
                nc.sync.dma_start(out_d[el, sl, :], ho[:])

    nc.compile()
    return nc


def _host_prep(inputs):
    g = {k: np.asarray(v, np.float32) for k, v in inputs.items()}
    sc1 = g["bn1_g"] / np.sqrt(g["bn1_v"] + EPS)
    sh1 = g["bn1_b"] - g["bn1_m"] * sc1
    sc2 = g["bn2_g"] / np.sqrt(g["bn2_v"] + EPS)
    sh2 = g["bn2_b"] - g["bn2_m"] * sc2

    w1p = g["conv1_w"][:, :, 0] * sc1[:, None]          # (O, I)
    w2p = g["conv2_w"] * sc2[:, None, None]             # (O, I, 17)

    def lhsT_pack(w):   # (O, I) -> (128, kc=I/128, mc=O/128, 128): [p,kc,mc,m]
        o, i = w.shape
        return np.ascontiguousarray(np.transpose(
            w.T.reshape(i // 128, 128, o // 128, 128), (1, 0, 2, 3)))

    w1t = lhsT_pack(w1p).astype(BF)
    w2t = np.stack([lhsT_pack(w2p[:, :, t]) for t in range(KT)], axis=2)
    w2t = np.ascontiguousarray(np.transpose(w2t, (0, 1, 2, 3, 4)))  # [p,kc,t,mc,m]
    w2t = w2t.astype(E4)

    def rhs_pack(wt):   # (Kdim, F) -> (128, kc, F)
        k, f = wt.shape
        return np.ascontiguousarray(
            np.transpose(wt.reshape(k // 128, 128, f), (1, 0, 2)))

    wmsgt = rhs_pack(g["w_msg"].T).astype(BF)
    wiht = rhs_pack(g["w_ih"].T).astype(BF)
    whh_t = g["w_hh"].T.copy()                           # (256, 768)
    whh_t[:, 2 * D:] *= 0.5                              # n-gate pre-scaled
    whht = rhs_pack(whh_t).astype(BF)

    wse_t = g["w_se"].T                                  # (256, 3)
    wse_hi = wse_t.astype(BF)
    wse_lo = (wse_t - wse_hi.astype(np.float32)).astype(BF)
    wsehl = rhs_pack(np.concatenate(
        [wse_hi.astype(np.float32), wse_lo.astype(np.float32)],
        axis=1)).astype(BF)

    bih, bhh = g["b_ih"], g["b_hh"]
    feed = {
        "w1t": w1t, "w2t": w2t,
        "sh1": np.ascontiguousarray(sh1.reshape(DC, 128).T.astype(np.float32)),
        "sh2": np.ascontiguousarray(sh2.reshape(DC, 128).T.astype(np.float32)),
        "wmsgt": wmsgt, "bmsg": g["b_msg"].reshape(1, D).astype(BF),
        "wsehl": wsehl,
        "bse": g["b_se"].reshape(1, 3).astype(BF),
        "wiht": wiht, "whht": whht,
        "brow_rz": (bih[:2 * D] + bhh[:2 * D]).reshape(1, 2 * D).astype(BF),
        "brow_gin": bih[2 * D:].reshape(1, D).astype(BF),
        "brow_ghn": (0.5 * bhh[2 * D:]).reshape(1, D).astype(BF),
        "ones128": np.ones((1, 128), BF),
        "ones512": np.ones((1, 512), BF),
        "ones2n": np.ones((2, N), BF),
    }
    return g, feed


def make_in_maps(inputs):
    g, feed = _host_prep(inputs)
    x = g["x"]
    mask = g["mask"]
    use_mask = not bool(np.all(mask == 1.0))
    use_bias = not (np.all(g["b_se"] == 0) and np.all(g["b_msg"] == 0)
                    and np.all(g["b_ih"] == 0) and np.all(g["b_hh"] == 0))
    in_maps = []
    for i in range(NCORE):
        m = dict(feed)
        m["x"] = np.ascontiguousarray(x[i * PER:(i + 1) * PER])
        if use_mask:
            m["maskt"] = np.ascontiguousarray(
                mask[i * PER:(i + 1) * PER].transpose(0, 2, 1)).astype(BF)
        in_maps.append(m)
    return in_maps, use_mask, use_bias


def get_nc(use_mask: bool, use_bias: bool = True):
    key = (use_mask, use_bias)
    if key not in _built:
        _built[key] = _build(use_mask, use_bias)
    return _built[key]


def kernel(**inputs) -> np.ndarray:
    in_maps, use_mask, use_bias = make_in_maps(inputs)
    nc = get_nc(use_mask, use_bias)
    from concourse import bass_utils
    last_err = None
    for attempt in range(3):
        try:
            res = bass_utils.run_bass_kernel_spmd(nc, in_maps,
                                                  core_ids=list(range(NCORE)))
            out = np.concatenate([res.results[i]["out"] for i in range(NCORE)],
                                 axis=0)
            return np.ascontiguousarray(out.astype(np.float32))
        except Exception as e:  # wedged device: reset backend and retry
            last_err = e
            try:
                import jax
                jax.clear_caches()
                jax.extend.backend.clear_backends()
            except Exception:
                pass
            import time as _t
            _t.sleep(5)
    raise last_err

